# revision 27
# baseline (speedup 1.0000x reference)
"""Multi-head attention (B=4, S=2048, D=1024, H=16, causal) on 8 trn2 cores.

Sharding: data-parallel over batch (4) x tensor-parallel over head groups (2).
Core c handles batch b=c//2, heads g=c%2 (8 heads each). Each core computes
its partial output projection; host sums the two partials per batch and adds
the bias.

Per-core pipeline (all matmul inputs fp16, fp32 accumulation). The q/k/v
projections are interleaved with the attention blocks (k/q emitted per
512-col chunk right before the four s-blocks that first need them, v per
256-col group during the early blocks) so the projection's PE work overlaps
the attention's DVE/ACT work. Attention per s-block i and local head h uses
an online two-part softmax so score psum tiles are freed early (psum is the
concurrency limiter), software-pipelined one block deep (chains of block i
are emitted before the tail of block i-1 so the PE stream never head-of-line
blocks on the tail's transposes):
  chains(i): scores via K=64 matmuls (causal mask added on the PE via an
    identity-weight matmul of a constant triangular NEG tile); row-max per
    part (DVE); exp(bias=-part max) with accum_out denominator (ACT) ->
    unnormalized E fp16; exact part B transposed to E^T immediately.
  tail(i): part A correction alpha = exp(mA - m) applied to E_A on the
    otherwise-idle gpsimd engine; den = alpha*accA + accB; part A
    transposed; PV in out[s,dk] orientation (po += E^T_j.T @ v_j);
    deferred normalization po * (1/den) fused into the psum->sbuf copy
    (DVE); concat -> concT via DMA transpose; y = concT.T @ Wo -> fp16.
"""

import math

import numpy as np

B, S, D, H = 4, 2048, 1024, 16
DK = 64
HLOC = 8          # heads per core
HD = HLOC * DK    # 512 local concat dims
P = 128
SBLKS = S // P    # 16
CH = 512          # score chunk width
KO = D // P       # 8 contraction tiles for projections
MPAIRS = 4        # head pairs per core
NEG = -30000.0


def build():
    import concourse.bass as bass
    import concourse.mybir as mybir
    import concourse.tile as tile
    from concourse import bacc

    fp16 = mybir.dt.float16
    f32 = mybir.dt.float32

    nc = bacc.Bacc()

    xtq = nc.dram_tensor("xtq", [D, S], fp16, kind="ExternalInput")
    xtk = nc.dram_tensor("xtk", [D, S], fp16, kind="ExternalInput")
    xtv = nc.dram_tensor("xtv", [D, S], fp16, kind="ExternalInput")
    wq = nc.dram_tensor("wq", [D, HD], fp16, kind="ExternalInput")
    wk = nc.dram_tensor("wk", [D, HD], fp16, kind="ExternalInput")
    wv = nc.dram_tensor("wv", [D, HD], fp16, kind="ExternalInput")
    wo = nc.dram_tensor("wo", [HD, D], fp16, kind="ExternalInput")
    trimask = nc.dram_tensor("trimask", [P, P], fp16, kind="ExternalInput")
    ident = nc.dram_tensor("ident", [P, P], fp16, kind="ExternalInput")
    y = nc.dram_tensor("y", [S, D], fp16, kind="ExternalOutput")

    with tile.TileContext(nc) as tc:
        with (
            tc.tile_pool(name="persist", bufs=1) as persist,
            tc.tile_pool(name="stats", bufs=2) as stats,
            tc.tile_pool(name="xkq", bufs=1) as xkqp,
            tc.tile_pool(name="xv", bufs=2) as xvp,
            tc.tile_pool(name="ebA", bufs=14) as ebAp,
            tc.tile_pool(name="ebB", bufs=4) as ebBp,
            tc.tile_pool(name="pts", bufs=2) as ptsp,
            tc.tile_pool(name="cc", bufs=2) as ccp,
            tc.tile_pool(name="psp", bufs=3, space="PSUM") as psp,
            tc.tile_pool(name="post", bufs=1, space="PSUM") as post,
        ):
            trimask_sb = persist.tile([P, P], fp16, tag="trimask")
            ident_sb = persist.tile([P, P], fp16, tag="ident")
            nc.sync.dma_start(out=trimask_sb, in_=trimask[:])
            nc.sync.dma_start(out=ident_sb, in_=ident[:])

            wo_sb = persist.tile([P, MPAIRS, D], fp16, tag="wo")
            nc.sync.dma_start(out=wo_sb, in_=wo[:].rearrange("(m p) n -> p m n", p=P))
            wq_sb = persist.tile([P, KO, HD], fp16, tag="wq")
            wk_sb = persist.tile([P, KO, HD], fp16, tag="wk")
            wv_sb = persist.tile([P, KO, HD], fp16, tag="wv")
            nc.sync.dma_start(out=wq_sb, in_=wq[:].rearrange("(ko p) n -> p ko n", p=P))
            nc.sync.dma_start(out=wk_sb, in_=wk[:].rearrange("(ko p) n -> p ko n", p=P))
            nc.sync.dma_start(out=wv_sb, in_=wv[:].rearrange("(ko p) n -> p ko n", p=P))

            # persistent activations
            qt = persist.tile([P, MPAIRS, S], fp16, tag="qt")   # rows = hd % 128
            kt = persist.tile([P, MPAIRS, S], fp16, tag="kt")
            vv = persist.tile([P, SBLKS, HD], fp16, tag="vv")   # [t%128, t//128, hd]

            xq_r = xtq[:].rearrange("(ko p) s -> p ko s", p=P)
            xk_r = xtk[:].rearrange("(ko p) s -> p ko s", p=P)
            xv_r = xtv[:].rearrange("(ko p) s -> p ko s", p=P)

            Exp = mybir.ActivationFunctionType.Exp
            X = mybir.AxisListType.X
            state = {}

            xc_tiles = {}

            def emit_kq_load(nch):
                """DMA one 512-col chunk of X_k / X_q."""
                for src_r, tg in ((xk_r, "xk"), (xq_r, "xq")):
                    xc = xkqp.tile([P, KO, CH], fp16, tag=tg,
                                   name=f"{tg}{nch}")
                    xc_tiles[(tg, nch)] = xc
                    nc.sync.dma_start(
                        out=xc, in_=src_r[:, :, nch * CH : (nch + 1) * CH]
                    )

            def emit_kq_group(nch, proj, m):
                """One m-group of the k or q projection for chunk nch."""
                wsb, dst, tg = ((wk_sb, kt, "xk") if proj == "k"
                                else (wq_sb, qt, "xq"))
                xc = xc_tiles[(tg, nch)]
                ps = psp.tile([P, 2 * CH], f32, tag="ps",
                              name=f"{tg}p{nch}_{m}")
                for ko in range(KO):
                    nc.tensor.matmul(
                        ps[:, :CH],
                        lhsT=wsb[:, ko, m * P : (m + 1) * P],
                        rhs=xc[:, ko, :],
                        start=(ko == 0),
                        stop=(ko == KO - 1),
                    )
                nc.scalar.copy(
                    out=dst[:, m, nch * CH : (nch + 1) * CH],
                    in_=ps[:, :CH],
                )

            def emit_v_proj(t2):
                """Project X_v group t2 (two 128-col t-blocks) -> vv."""
                xc = xvp.tile([P, KO, 2 * P], fp16, tag="xv", name=f"xv{t2}")
                nc.sync.dma_start(
                    out=xc, in_=xv_r[:, :, t2 * 2 * P : (t2 + 1) * 2 * P]
                )
                ps = psp.tile([P, 2 * CH], f32, tag="ps", name=f"vp{t2}")
                for half in range(2):
                    for ko in range(KO):
                        nc.tensor.matmul(
                            ps[:, half * CH : (half + 1) * CH],
                            lhsT=xc[:, ko, half * P : (half + 1) * P],
                            rhs=wv_sb[:, ko, :],
                            start=(ko == 0),
                            stop=(ko == KO - 1),
                        )
                eng = nc.vector.tensor_copy if t2 % 2 == 0 else nc.scalar.copy
                eng(
                    out=vv[:, t2 * 2 : t2 * 2 + 2, :].rearrange("p a b -> p (a b)"),
                    in_=ps,
                )

            def emit_chains(i):
                c, r = i // 4, i % 4
                width = (i + 1) * P
                late = c >= 2
                wA = min(width, 2 * CH)
                wB = width - wA
                st = {"late": late, "wA": wA, "wB": wB, "pts": [], "ebuf": []}
                st["den"] = stats.tile([P, HLOC], f32, tag="den", name=f"den{i}")
                st["negmA"] = stats.tile([P, HLOC], f32, tag="negmA",
                                         name=f"negmA{i}")
                if late:
                    for t in ("negmB", "negm", "accA", "accB", "alpha"):
                        st[t] = stats.tile([P, HLOC], f32, tag=t, name=f"{t}{i}")
                for h in range(HLOC):
                    m, z = h // 2, h % 2
                    qts = qt[z * DK : (z + 1) * DK, m, i * P : (i + 1) * P]
                    tA = psp.tile([P, 2 * CH], f32, tag="ps", name=f"sA{i}_{h}")
                    for cc in range(min(c, 1) + 1):
                        w = CH if cc < c else (r + 1) * P
                        nc.tensor.matmul(
                            tA[:, cc * CH : cc * CH + w],
                            lhsT=qts,
                            rhs=kt[z * DK : (z + 1) * DK, m, cc * CH : cc * CH + w],
                            start=True,
                            stop=(cc != c),
                        )
                    if not late:
                        nc.tensor.matmul(
                            tA[:, c * CH + r * P : c * CH + (r + 1) * P],
                            lhsT=ident_sb, rhs=trimask_sb,
                            start=False, stop=True,
                        )
                    nc.vector.reduce_max(
                        st["negmA"][:, h : h + 1], tA[:, :wA], axis=X,
                        negate=True,
                    )
                    ebuf = ebAp.tile([P, 2 * CH], fp16, tag="eb",
                                     name=f"eb{i}_{h}")
                    st["ebuf"].append(ebuf)
                    pt = ptsp.tile([P, SBLKS, P], fp16, tag=f"pt{h}",
                                   name=f"pt{i}_{h}")
                    st["pts"].append(pt)
                    nc.scalar.activation(
                        out=ebuf[:, :wA], in_=tA[:, :wA], func=Exp,
                        bias=st["negmA"][:, h : h + 1], scale=1.0,
                        accum_out=(st["accA"] if late else st["den"])[:, h : h + 1],
                    )
                    if not late:
                        nc.sync.dma_start(
                            out=pt[:, 0 : i + 1, :], in_=ebuf[:, 0:width],
                            transpose=True,
                        )
                        continue
                    tB = psp.tile([P, 2 * CH], f32, tag="ps", name=f"sB{i}_{h}")
                    for cc in range(2, c + 1):
                        w = CH if cc < c else (r + 1) * P
                        nc.tensor.matmul(
                            tB[:, (cc - 2) * CH : (cc - 2) * CH + w],
                            lhsT=qts,
                            rhs=kt[z * DK : (z + 1) * DK, m, cc * CH : cc * CH + w],
                            start=True,
                            stop=(cc != c),
                        )
                    nc.tensor.matmul(
                        tB[:, (c - 2) * CH + r * P : (c - 2) * CH + (r + 1) * P],
                        lhsT=ident_sb, rhs=trimask_sb,
                        start=False, stop=True,
                    )
                    nc.vector.reduce_max(
                        st["negmB"][:, h : h + 1], tB[:, :wB], axis=X,
                        negate=True,
                    )
                    # negm = -max(mA, mB) = min(negmA, negmB)
                    nc.gpsimd.tensor_tensor(
                        out=st["negm"][:, h : h + 1],
                        in0=st["negmA"][:, h : h + 1],
                        in1=st["negmB"][:, h : h + 1], op=mybir.AluOpType.min,
                    )
                    ebB = ebBp.tile([P, 2 * CH], fp16, tag="ebB",
                                    name=f"ebB{i}_{h}")
                    nc.scalar.activation(
                        out=ebB[:, :wB], in_=tB[:, :wB], func=Exp,
                        bias=st["negm"][:, h : h + 1], scale=1.0,
                        accum_out=st["accB"][:, h : h + 1],
                    )
                    # exact part B transposed now; part A in the tail
                    nc.sync.dma_start(
                        out=pt[:, 8 : i + 1, :], in_=ebB[:, :wB],
                        transpose=True,
                    )
                state[i] = st

            def emit_tail(i):
                st = state.pop(i)
                invden = stats.tile([P, HLOC], f32, tag="invden",
                                    name=f"invden{i}")
                if st["late"]:
                    # alpha = exp(mA - m) = exp(negm - negmA), batched [P, 8]
                    dmx = stats.tile([P, HLOC], f32, tag="dmx", name=f"dmx{i}")
                    nc.gpsimd.tensor_tensor(
                        out=dmx, in0=st["negm"], in1=st["negmA"],
                        op=mybir.AluOpType.subtract,
                    )
                    nc.scalar.activation(out=st["alpha"], in_=dmx, func=Exp,
                                         bias=0.0, scale=1.0)
                    # den = alpha * accA + accB
                    nc.gpsimd.tensor_tensor(out=st["accA"], in0=st["accA"],
                                            in1=st["alpha"],
                                            op=mybir.AluOpType.mult)
                    nc.gpsimd.tensor_tensor(out=st["den"], in0=st["accA"],
                                            in1=st["accB"],
                                            op=mybir.AluOpType.add)
                    for h in range(HLOC):
                        nc.vector.tensor_scalar(
                            out=st["ebuf"][h][:, : st["wA"]],
                            in0=st["ebuf"][h][:, : st["wA"]],
                            scalar1=st["alpha"][:, h : h + 1],
                            scalar2=None,
                            op0=mybir.AluOpType.mult,
                        )
                        nc.sync.dma_start(
                            out=st["pts"][h][:, 0:8, :],
                            in_=st["ebuf"][h][:, : st["wA"]],
                            transpose=True,
                        )
                nc.vector.reciprocal(invden, st["den"])

                # PV: po[s, h*64:(h+1)*64] = sum_j E^T_j.T @ v_j
                po = post.tile([P, HD], f32, tag="po", name=f"po{i}")
                for h in range(HLOC):
                    for j in range(i + 1):
                        nc.tensor.matmul(
                            po[:, h * DK : (h + 1) * DK],
                            lhsT=st["pts"][h][:, j, :],
                            rhs=vv[:, j, h * DK : (h + 1) * DK],
                            start=(j == 0),
                            stop=(j == i),
                        )
                # normalize + copy to sbuf in one DVE op
                conc = ccp.tile([P, HD], fp16, tag="conc", name=f"conc{i}")
                nc.vector.tensor_tensor(
                    out=conc[:, :].rearrange("p (h k) -> p h k", h=HLOC),
                    in0=po[:, :].rearrange("p (h k) -> p h k", h=HLOC),
                    in1=invden[:, :, None].broadcast_to((P, HLOC, DK)),
                    op=mybir.AluOpType.mult,
                )
                # conc[s, hd] -> concT[hd%128, m, s-block]
                concT = ccp.tile([P, MPAIRS, P], fp16, tag="concT",
                                 name=f"concT{i}")
                nc.sync.dma_start(out=concT, in_=conc[:, :], transpose=True)
                # output projection for this s-block
                ysb = ccp.tile([P, D], fp16, tag="ysb", name=f"ysb{i}")
                for nch in range(2):
                    ypt = post.tile([P, CH], f32, tag="yp", name=f"yp{i}_{nch}")
                    for m in range(MPAIRS):
                        nc.tensor.matmul(
                            ypt,
                            lhsT=concT[:, m, :],
                            rhs=wo_sb[:, m, nch * CH : (nch + 1) * CH],
                            start=(m == 0),
                            stop=(m == MPAIRS - 1),
                        )
                    nc.scalar.copy(
                        out=ysb[:, nch * CH : (nch + 1) * CH], in_=ypt
                    )
                nc.sync.dma_start(out=y[:][i * P : (i + 1) * P, :], in_=ysb)

            # Block processing order: 1..15 then 0 so the final (drain) tail
            # is the smallest block. Projection chunk work is spread across
            # steps: chunk 0 fully at step 0; chunk nch>=1 split over the
            # three steps before the first block that needs it completes.
            blk_order = list(range(1, 12)) + [15, 14, 13, 12, 0]
            proj_work = {s: [] for s in range(SBLKS)}
            proj_work[0] = [("load", 0)] + [
                ("grp", 0, p, m) for p in ("k", "q") for m in range(MPAIRS)
            ]
            for nch in range(1, 4):
                groups = [("grp", nch, p, m) for p in ("k", "q")
                          for m in range(MPAIRS)]
                base = 4 * (nch - 1)
                proj_work[base].append(("load", nch))
                for gi, g in enumerate(groups):
                    proj_work[base + (gi % 3)].append(g)

            for s in range(SBLKS + 1):
                if s < SBLKS:
                    for w in proj_work[s]:
                        if w[0] == "load":
                            emit_kq_load(w[1])
                        else:
                            emit_kq_group(w[1], w[2], w[3])
                    if s < 8:
                        emit_v_proj(s)
                    emit_chains(blk_order[s])
                if s >= 1:
                    emit_tail(blk_order[s - 1])

    nc.finalize()
    return nc


def _prep_inputs(Q, K, V, Wq, Wk, Wv, Wo):
    """Host-side shard + layout prep. Returns list of 8 in_maps."""
    rt8 = math.sqrt(math.sqrt(64.0))  # sqrt(8): scale split over q and k
    tri = np.where(
        np.arange(P)[None, :] <= np.arange(P)[:, None], 0.0, NEG
    ).astype(np.float16)
    ident = np.eye(P, dtype=np.float16)
    in_maps = []
    for c in range(8):
        b, g = c // 2, c % 2
        heads = slice(g * HLOC, (g + 1) * HLOC)
        wq_p = (Wq[heads] * rt8).transpose(1, 0, 2).reshape(D, HD)
        wk_p = (Wk[heads] * rt8).transpose(1, 0, 2).reshape(D, HD)
        wv_p = Wv[heads].transpose(1, 0, 2).reshape(D, HD)
        wo_p = Wo[:, g * HD : (g + 1) * HD].T  # [HD, D]
        in_maps.append({
            "xtq": np.ascontiguousarray(Q[b].T).astype(np.float16),
            "xtk": np.ascontiguousarray(K[b].T).astype(np.float16),
            "xtv": np.ascontiguousarray(V[b].T).astype(np.float16),
            "wq": np.ascontiguousarray(wq_p).astype(np.float16),
            "wk": np.ascontiguousarray(wk_p).astype(np.float16),
            "wv": np.ascontiguousarray(wv_p).astype(np.float16),
            "wo": np.ascontiguousarray(wo_p).astype(np.float16),
            "trimask": tri,
            "ident": ident,
        })
    return in_maps


_NC = []


def kernel(Q, K, V, mask, Wq, Wk, Wv, Wo, bo, _trace=False):
    from concourse.bass_utils import run_bass_kernel_spmd

    Q, K, V = np.asarray(Q), np.asarray(K), np.asarray(V)
    Wq, Wk, Wv = np.asarray(Wq), np.asarray(Wk), np.asarray(Wv)
    Wo, bo = np.asarray(Wo), np.asarray(bo)

    if not _NC:
        _NC.append(build())
    nc = _NC[0]
    in_maps = _prep_inputs(Q, K, V, Wq, Wk, Wv, Wo)
    res = run_bass_kernel_spmd(nc, in_maps, core_ids=list(range(8)), trace=_trace)
    ys = [r["y"].astype(np.float32) for r in res.results]
    out = np.stack([ys[2 * b] + ys[2 * b + 1] for b in range(B)])
    out = out + bo[None, None, :].astype(np.float32)
    if _trace:
        kernel._last = res
    return out.astype(np.float32)


# revision 41
# speedup vs baseline: 1.0415x; 1.0415x over previous
"""Multi-head attention (B=4, S=2048, D=1024, H=16, causal) on 8 trn2 cores.

Sharding: data-parallel over batch (4) x tensor-parallel over head groups (2).
Core c handles batch b=c//2, heads g=c%2 (8 heads each). Each core computes
its partial output projection; host sums the two partials per batch and adds
the bias.

Per-core pipeline (all matmul inputs fp16, fp32 accumulation). The q/k/v
projections are interleaved with the attention blocks (k/q emitted per
512-col chunk right before the four s-blocks that first need them, v per
256-col group during the early blocks) so the projection's PE work overlaps
the attention's DVE/ACT work. Attention per s-block i and local head h uses
an online two-part softmax so score psum tiles are freed early (psum is the
concurrency limiter), software-pipelined one block deep (chains of block i
are emitted before the tail of block i-1 so the PE stream never head-of-line
blocks on the tail's transposes):
  chains(i): scores via K=64 matmuls (causal mask added on the PE via an
    identity-weight matmul of a constant triangular NEG tile); row-max per
    part (DVE); exp(bias=-part max) with accum_out denominator (ACT) ->
    unnormalized E fp16; exact part B transposed to E^T immediately.
  tail(i): part A correction alpha = exp(mA - m) applied to E_A on the
    otherwise-idle gpsimd engine; den = alpha*accA + accB; part A
    transposed; PV in out[s,dk] orientation (po += E^T_j.T @ v_j);
    deferred normalization po * (1/den) fused into the psum->sbuf copy
    (DVE); concat -> concT via DMA transpose; y = concT.T @ Wo -> fp16.
"""

import math

import numpy as np

B, S, D, H = 4, 2048, 1024, 16
DK = 64
HLOC = 8          # heads per core
HD = HLOC * DK    # 512 local concat dims
P = 128
SBLKS = S // P    # 16
CH = 512          # score chunk width
KO = D // P       # 8 contraction tiles for projections
MPAIRS = 4        # head pairs per core
NEG = -30000.0

# schedule/buffer knobs (module-level so they can be tuned)
BLK_ORDER = [2, 3, 4, 5, 6, 7, 8, 9, 10, 11, 15, 14, 13, 12, 0, 1]
EBA_BUFS = 11
EBB_BUFS = 4
PROJ_SPLIT = 2


def build():
    import concourse.bass as bass
    import concourse.mybir as mybir
    import concourse.tile as tile
    from concourse import bacc

    fp16 = mybir.dt.float16
    f32 = mybir.dt.float32

    nc = bacc.Bacc()

    xtq = nc.dram_tensor("xtq", [D, S], fp16, kind="ExternalInput")
    xtk = nc.dram_tensor("xtk", [D, S], fp16, kind="ExternalInput")
    xtv = nc.dram_tensor("xtv", [D, S], fp16, kind="ExternalInput")
    wq = nc.dram_tensor("wq", [D, HD], fp16, kind="ExternalInput")
    wk = nc.dram_tensor("wk", [D, HD], fp16, kind="ExternalInput")
    wv = nc.dram_tensor("wv", [D, HD], fp16, kind="ExternalInput")
    wo = nc.dram_tensor("wo", [HD, D], fp16, kind="ExternalInput")
    trimask = nc.dram_tensor("trimask", [P, P], fp16, kind="ExternalInput")
    ident = nc.dram_tensor("ident", [P, P], fp16, kind="ExternalInput")
    y = nc.dram_tensor("y", [S, D], fp16, kind="ExternalOutput")

    with tile.TileContext(nc) as tc:
        with (
            tc.tile_pool(name="persist", bufs=1) as persist,
            tc.tile_pool(name="stats", bufs=2) as stats,
            tc.tile_pool(name="xkq", bufs=1) as xkqp,
            tc.tile_pool(name="xv", bufs=2) as xvp,
            tc.tile_pool(name="ebA", bufs=EBA_BUFS) as ebAp,
            tc.tile_pool(name="ebB", bufs=EBB_BUFS) as ebBp,
            tc.tile_pool(name="pts", bufs=2) as ptsp,
            tc.tile_pool(name="cc", bufs=2) as ccp,
            tc.tile_pool(name="psp", bufs=3, space="PSUM") as psp,
            tc.tile_pool(name="post", bufs=1, space="PSUM") as post,
        ):
            trimask_sb = persist.tile([P, P], fp16, tag="trimask")
            ident_sb = persist.tile([P, P], fp16, tag="ident")
            nc.sync.dma_start(out=trimask_sb, in_=trimask[:])
            nc.sync.dma_start(out=ident_sb, in_=ident[:])

            wo_sb = persist.tile([P, MPAIRS, D], fp16, tag="wo")
            nc.sync.dma_start(out=wo_sb, in_=wo[:].rearrange("(m p) n -> p m n", p=P))
            wq_sb = persist.tile([P, KO, HD], fp16, tag="wq")
            wk_sb = persist.tile([P, KO, HD], fp16, tag="wk")
            wv_sb = persist.tile([P, KO, HD], fp16, tag="wv")
            nc.sync.dma_start(out=wq_sb, in_=wq[:].rearrange("(ko p) n -> p ko n", p=P))
            nc.sync.dma_start(out=wk_sb, in_=wk[:].rearrange("(ko p) n -> p ko n", p=P))
            nc.sync.dma_start(out=wv_sb, in_=wv[:].rearrange("(ko p) n -> p ko n", p=P))

            # persistent activations
            qt = persist.tile([P, MPAIRS, S], fp16, tag="qt")   # rows = hd % 128
            kt = persist.tile([P, MPAIRS, S], fp16, tag="kt")
            vv = persist.tile([P, SBLKS, HD], fp16, tag="vv")   # [t%128, t//128, hd]

            xq_r = xtq[:].rearrange("(ko p) s -> p ko s", p=P)
            xk_r = xtk[:].rearrange("(ko p) s -> p ko s", p=P)
            xv_r = xtv[:].rearrange("(ko p) s -> p ko s", p=P)

            Exp = mybir.ActivationFunctionType.Exp
            X = mybir.AxisListType.X
            state = {}

            xc_tiles = {}

            def emit_kq_load(nch):
                """DMA one 512-col chunk of X_k / X_q."""
                for src_r, tg in ((xk_r, "xk"), (xq_r, "xq")):
                    xc = xkqp.tile([P, KO, CH], fp16, tag=tg,
                                   name=f"{tg}{nch}")
                    xc_tiles[(tg, nch)] = xc
                    nc.sync.dma_start(
                        out=xc, in_=src_r[:, :, nch * CH : (nch + 1) * CH]
                    )

            def emit_kq_group(nch, proj, m):
                """One m-group of the k or q projection for chunk nch."""
                wsb, dst, tg = ((wk_sb, kt, "xk") if proj == "k"
                                else (wq_sb, qt, "xq"))
                xc = xc_tiles[(tg, nch)]
                ps = psp.tile([P, 2 * CH], f32, tag="ps",
                              name=f"{tg}p{nch}_{m}")
                for ko in range(KO):
                    nc.tensor.matmul(
                        ps[:, :CH],
                        lhsT=wsb[:, ko, m * P : (m + 1) * P],
                        rhs=xc[:, ko, :],
                        start=(ko == 0),
                        stop=(ko == KO - 1),
                    )
                nc.scalar.copy(
                    out=dst[:, m, nch * CH : (nch + 1) * CH],
                    in_=ps[:, :CH],
                )

            def emit_v_proj(t2):
                """Project X_v group t2 (two 128-col t-blocks) -> vv."""
                xc = xvp.tile([P, KO, 2 * P], fp16, tag="xv", name=f"xv{t2}")
                nc.sync.dma_start(
                    out=xc, in_=xv_r[:, :, t2 * 2 * P : (t2 + 1) * 2 * P]
                )
                ps = psp.tile([P, 2 * CH], f32, tag="ps", name=f"vp{t2}")
                for half in range(2):
                    for ko in range(KO):
                        nc.tensor.matmul(
                            ps[:, half * CH : (half + 1) * CH],
                            lhsT=xc[:, ko, half * P : (half + 1) * P],
                            rhs=wv_sb[:, ko, :],
                            start=(ko == 0),
                            stop=(ko == KO - 1),
                        )
                eng = nc.vector.tensor_copy if t2 % 2 == 0 else nc.scalar.copy
                eng(
                    out=vv[:, t2 * 2 : t2 * 2 + 2, :].rearrange("p a b -> p (a b)"),
                    in_=ps,
                )

            def emit_chains(i):
                c, r = i // 4, i % 4
                width = (i + 1) * P
                late = c >= 2
                wA = min(width, 2 * CH)
                wB = width - wA
                st = {"late": late, "wA": wA, "wB": wB, "pts": [], "ebuf": []}
                st["den"] = stats.tile([P, HLOC], f32, tag="den", name=f"den{i}")
                st["negmA"] = stats.tile([P, HLOC], f32, tag="negmA",
                                         name=f"negmA{i}")
                if late:
                    for t in ("negmB", "negm", "accA", "accB", "alpha"):
                        st[t] = stats.tile([P, HLOC], f32, tag=t, name=f"{t}{i}")
                for h in range(HLOC):
                    m, z = h // 2, h % 2
                    qts = qt[z * DK : (z + 1) * DK, m, i * P : (i + 1) * P]
                    tA = psp.tile([P, 2 * CH], f32, tag="ps", name=f"sA{i}_{h}")
                    for cc in range(min(c, 1) + 1):
                        w = CH if cc < c else (r + 1) * P
                        nc.tensor.matmul(
                            tA[:, cc * CH : cc * CH + w],
                            lhsT=qts,
                            rhs=kt[z * DK : (z + 1) * DK, m, cc * CH : cc * CH + w],
                            start=True,
                            stop=(cc != c),
                        )
                    if not late:
                        nc.tensor.matmul(
                            tA[:, c * CH + r * P : c * CH + (r + 1) * P],
                            lhsT=ident_sb, rhs=trimask_sb,
                            start=False, stop=True,
                        )
                    nc.vector.reduce_max(
                        st["negmA"][:, h : h + 1], tA[:, :wA], axis=X,
                        negate=True,
                    )
                    ebuf = ebAp.tile([P, 2 * CH], fp16, tag="eb",
                                     name=f"eb{i}_{h}")
                    st["ebuf"].append(ebuf)
                    pt = ptsp.tile([P, SBLKS, P], fp16, tag=f"pt{h}",
                                   name=f"pt{i}_{h}")
                    st["pts"].append(pt)
                    nc.scalar.activation(
                        out=ebuf[:, :wA], in_=tA[:, :wA], func=Exp,
                        bias=st["negmA"][:, h : h + 1], scale=1.0,
                        accum_out=(st["accA"] if late else st["den"])[:, h : h + 1],
                    )
                    if not late:
                        nc.sync.dma_start(
                            out=pt[:, 0 : i + 1, :], in_=ebuf[:, 0:width],
                            transpose=True,
                        )
                        continue
                    tB = psp.tile([P, 2 * CH], f32, tag="ps", name=f"sB{i}_{h}")
                    for cc in range(2, c + 1):
                        w = CH if cc < c else (r + 1) * P
                        nc.tensor.matmul(
                            tB[:, (cc - 2) * CH : (cc - 2) * CH + w],
                            lhsT=qts,
                            rhs=kt[z * DK : (z + 1) * DK, m, cc * CH : cc * CH + w],
                            start=True,
                            stop=(cc != c),
                        )
                    nc.tensor.matmul(
                        tB[:, (c - 2) * CH + r * P : (c - 2) * CH + (r + 1) * P],
                        lhsT=ident_sb, rhs=trimask_sb,
                        start=False, stop=True,
                    )
                    nc.vector.reduce_max(
                        st["negmB"][:, h : h + 1], tB[:, :wB], axis=X,
                        negate=True,
                    )
                    # negm = -max(mA, mB) = min(negmA, negmB)
                    nc.vector.tensor_tensor(
                        out=st["negm"][:, h : h + 1],
                        in0=st["negmA"][:, h : h + 1],
                        in1=st["negmB"][:, h : h + 1], op=mybir.AluOpType.min,
                    )
                    ebB = ebBp.tile([P, 2 * CH], fp16, tag="ebB",
                                    name=f"ebB{i}_{h}")
                    nc.scalar.activation(
                        out=ebB[:, :wB], in_=tB[:, :wB], func=Exp,
                        bias=st["negm"][:, h : h + 1], scale=1.0,
                        accum_out=st["accB"][:, h : h + 1],
                    )
                    # exact part B transposed now; part A in the tail
                    nc.sync.dma_start(
                        out=pt[:, 8 : i + 1, :], in_=ebB[:, :wB],
                        transpose=True,
                    )
                state[i] = st

            def emit_tail(i):
                st = state.pop(i)
                invden = stats.tile([P, HLOC], f32, tag="invden",
                                    name=f"invden{i}")
                if st["late"]:
                    # alpha = exp(mA - m) = exp(negm - negmA), batched [P, 8]
                    dmx = stats.tile([P, HLOC], f32, tag="dmx", name=f"dmx{i}")
                    nc.vector.tensor_tensor(
                        out=dmx, in0=st["negm"], in1=st["negmA"],
                        op=mybir.AluOpType.subtract,
                    )
                    nc.scalar.activation(out=st["alpha"], in_=dmx, func=Exp,
                                         bias=0.0, scale=1.0)
                    # den = alpha * accA + accB
                    nc.vector.tensor_tensor(out=st["accA"], in0=st["accA"],
                                            in1=st["alpha"],
                                            op=mybir.AluOpType.mult)
                    nc.vector.tensor_tensor(out=st["den"], in0=st["accA"],
                                            in1=st["accB"],
                                            op=mybir.AluOpType.add)
                    for h in range(HLOC):
                        nc.vector.tensor_scalar(
                            out=st["ebuf"][h][:, : st["wA"]],
                            in0=st["ebuf"][h][:, : st["wA"]],
                            scalar1=st["alpha"][:, h : h + 1],
                            scalar2=None,
                            op0=mybir.AluOpType.mult,
                        )
                        nc.sync.dma_start(
                            out=st["pts"][h][:, 0:8, :],
                            in_=st["ebuf"][h][:, : st["wA"]],
                            transpose=True,
                        )
                nc.vector.reciprocal(invden, st["den"])

                # PV: po[s, h*64:(h+1)*64] = sum_j E^T_j.T @ v_j
                po = post.tile([P, HD], f32, tag="po", name=f"po{i}")
                for h in range(HLOC):
                    for j in range(i + 1):
                        nc.tensor.matmul(
                            po[:, h * DK : (h + 1) * DK],
                            lhsT=st["pts"][h][:, j, :],
                            rhs=vv[:, j, h * DK : (h + 1) * DK],
                            start=(j == 0),
                            stop=(j == i),
                        )
                # normalize + copy to sbuf in one DVE op
                conc = ccp.tile([P, HD], fp16, tag="conc", name=f"conc{i}")
                nc.vector.tensor_tensor(
                    out=conc[:, :].rearrange("p (h k) -> p h k", h=HLOC),
                    in0=po[:, :].rearrange("p (h k) -> p h k", h=HLOC),
                    in1=invden[:, :, None].broadcast_to((P, HLOC, DK)),
                    op=mybir.AluOpType.mult,
                )
                # conc[s, hd] -> concT[hd%128, m, s-block]
                concT = ccp.tile([P, MPAIRS, P], fp16, tag="concT",
                                 name=f"concT{i}")
                nc.sync.dma_start(out=concT, in_=conc[:, :], transpose=True)
                # output projection for this s-block
                ysb = ccp.tile([P, D], fp16, tag="ysb", name=f"ysb{i}")
                for nch in range(2):
                    ypt = post.tile([P, CH], f32, tag="yp", name=f"yp{i}_{nch}")
                    for m in range(MPAIRS):
                        nc.tensor.matmul(
                            ypt,
                            lhsT=concT[:, m, :],
                            rhs=wo_sb[:, m, nch * CH : (nch + 1) * CH],
                            start=(m == 0),
                            stop=(m == MPAIRS - 1),
                        )
                    nc.scalar.copy(
                        out=ysb[:, nch * CH : (nch + 1) * CH], in_=ypt
                    )
                nc.sync.dma_start(out=y[:][i * P : (i + 1) * P, :], in_=ysb)

            # Block processing order: 1..15 then 0 so the final (drain) tail
            # is the smallest block. Projection chunk work is spread across
            # steps: chunk 0 fully at step 0; chunk nch>=1 split over the
            # three steps before the first block that needs it completes.
            blk_order = list(BLK_ORDER)
            proj_work = {s: [] for s in range(SBLKS)}
            proj_work[0] = [("load", 0)] + [
                ("grp", 0, p, m) for p in ("k", "q") for m in range(MPAIRS)
            ]
            for nch in range(1, 4):
                groups = [("grp", nch, p, m) for p in ("k", "q")
                          for m in range(MPAIRS)]
                base = 4 * (nch - 1)
                proj_work[base].append(("load", nch))
                for gi, g in enumerate(groups):
                    proj_work[base + (gi % PROJ_SPLIT)].append(g)

            for s in range(SBLKS + 1):
                if s < SBLKS:
                    for w in proj_work[s]:
                        if w[0] == "load":
                            emit_kq_load(w[1])
                        else:
                            emit_kq_group(w[1], w[2], w[3])
                    if s < 8:
                        emit_v_proj(s)
                    emit_chains(blk_order[s])
                if s >= 1:
                    emit_tail(blk_order[s - 1])

    nc.finalize()
    return nc


def _prep_inputs(Q, K, V, Wq, Wk, Wv, Wo):
    """Host-side shard + layout prep. Returns list of 8 in_maps."""
    rt8 = math.sqrt(math.sqrt(64.0))  # sqrt(8): scale split over q and k
    tri = np.where(
        np.arange(P)[None, :] <= np.arange(P)[:, None], 0.0, NEG
    ).astype(np.float16)
    ident = np.eye(P, dtype=np.float16)
    in_maps = []
    for c in range(8):
        b, g = c // 2, c % 2
        heads = slice(g * HLOC, (g + 1) * HLOC)
        wq_p = (Wq[heads] * rt8).transpose(1, 0, 2).reshape(D, HD)
        wk_p = (Wk[heads] * rt8).transpose(1, 0, 2).reshape(D, HD)
        wv_p = Wv[heads].transpose(1, 0, 2).reshape(D, HD)
        wo_p = Wo[:, g * HD : (g + 1) * HD].T  # [HD, D]
        in_maps.append({
            "xtq": np.ascontiguousarray(Q[b].T).astype(np.float16),
            "xtk": np.ascontiguousarray(K[b].T).astype(np.float16),
            "xtv": np.ascontiguousarray(V[b].T).astype(np.float16),
            "wq": np.ascontiguousarray(wq_p).astype(np.float16),
            "wk": np.ascontiguousarray(wk_p).astype(np.float16),
            "wv": np.ascontiguousarray(wv_p).astype(np.float16),
            "wo": np.ascontiguousarray(wo_p).astype(np.float16),
            "trimask": tri,
            "ident": ident,
        })
    return in_maps


_NC = []


def kernel(Q, K, V, mask, Wq, Wk, Wv, Wo, bo, _trace=False):
    from concourse.bass_utils import run_bass_kernel_spmd

    Q, K, V = np.asarray(Q), np.asarray(K), np.asarray(V)
    Wq, Wk, Wv = np.asarray(Wq), np.asarray(Wk), np.asarray(Wv)
    Wo, bo = np.asarray(Wo), np.asarray(bo)

    if not _NC:
        _NC.append(build())
    nc = _NC[0]
    in_maps = _prep_inputs(Q, K, V, Wq, Wk, Wv, Wo)
    res = run_bass_kernel_spmd(nc, in_maps, core_ids=list(range(8)), trace=_trace)
    ys = [r["y"].astype(np.float32) for r in res.results]
    out = np.stack([ys[2 * b] + ys[2 * b + 1] for b in range(B)])
    out = out + bo[None, None, :].astype(np.float32)
    if _trace:
        kernel._last = res
    return out.astype(np.float32)


# revision 44
# speedup vs baseline: 1.0541x; 1.0121x over previous
"""Multi-head attention (B=4, S=2048, D=1024, H=16, causal) on 8 trn2 cores.

Sharding: data-parallel over batch (4) x tensor-parallel over head groups (2).
Core c handles batch b=c//2, heads g=c%2 (8 heads each). Each core computes
its partial output projection; host sums the two partials per batch and adds
the bias.

Per-core pipeline (all matmul inputs fp16, fp32 accumulation). The q/k/v
projections are interleaved with the attention blocks (k/q emitted per
512-col chunk right before the four s-blocks that first need them, v per
256-col group during the early blocks) so the projection's PE work overlaps
the attention's DVE/ACT work. Attention per s-block i and local head h uses
an online two-part softmax so score psum tiles are freed early (psum is the
concurrency limiter), software-pipelined one block deep (chains of block i
are emitted before the tail of block i-1 so the PE stream never head-of-line
blocks on the tail's transposes):
  chains(i): scores via K=64 matmuls (causal mask added on the PE via an
    identity-weight matmul of a constant triangular NEG tile); row-max per
    part (DVE); exp(bias=-part max) with accum_out denominator (ACT) ->
    unnormalized E fp16; exact part B transposed to E^T immediately.
  tail(i): part A correction alpha = exp(mA - m) applied to E_A via a
    4x-rate fp16 DVE tensor_scalar; den = alpha*accA + accB; part A
    transposed; PV in out[s,dk] orientation (po += E^T_j.T @ v_j);
    deferred normalization po * (1/den) fused into the psum->sbuf copy
    (DVE); concat -> concT via DMA transpose; y = concT.T @ Wo -> fp16.
Block processing order and the projection-group emission schedule
(BLK_ORDER / PROJ_STEPS) are tuned against the Tile cost model: big-block
tails overlap big-block chains and the final drain blocks are the small
ones.
"""

import math

import numpy as np

B, S, D, H = 4, 2048, 1024, 16
DK = 64
HLOC = 8          # heads per core
HD = HLOC * DK    # 512 local concat dims
P = 128
SBLKS = S // P    # 16
CH = 512          # score chunk width
KO = D // P       # 8 contraction tiles for projections
MPAIRS = 4        # head pairs per core
NEG = -30000.0

# schedule/buffer knobs (module-level so they can be tuned)
BLK_ORDER = [2, 3, 4, 5, 6, 7, 8, 9, 10, 11, 15, 14, 13, 12, 0, 1]
EBA_BUFS = 11
EBB_BUFS = 4
# per-chunk list of steps at which the 8 (k,q) m-groups are emitted
PROJ_STEPS = {1: [0, 0, 0, 1, 1, 1, 1, 1],
              2: [2, 2, 3, 3, 4, 4, 5, 5],
              3: [5, 6, 6, 7, 7, 8, 8, 9]}


def build():
    import concourse.bass as bass
    import concourse.mybir as mybir
    import concourse.tile as tile
    from concourse import bacc

    fp16 = mybir.dt.float16
    f32 = mybir.dt.float32

    nc = bacc.Bacc()

    xtq = nc.dram_tensor("xtq", [D, S], fp16, kind="ExternalInput")
    xtk = nc.dram_tensor("xtk", [D, S], fp16, kind="ExternalInput")
    xtv = nc.dram_tensor("xtv", [D, S], fp16, kind="ExternalInput")
    wq = nc.dram_tensor("wq", [D, HD], fp16, kind="ExternalInput")
    wk = nc.dram_tensor("wk", [D, HD], fp16, kind="ExternalInput")
    wv = nc.dram_tensor("wv", [D, HD], fp16, kind="ExternalInput")
    wo = nc.dram_tensor("wo", [HD, D], fp16, kind="ExternalInput")
    trimask = nc.dram_tensor("trimask", [P, P], fp16, kind="ExternalInput")
    ident = nc.dram_tensor("ident", [P, P], fp16, kind="ExternalInput")
    y = nc.dram_tensor("y", [S, D], fp16, kind="ExternalOutput")

    with tile.TileContext(nc) as tc:
        with (
            tc.tile_pool(name="persist", bufs=1) as persist,
            tc.tile_pool(name="stats", bufs=2) as stats,
            tc.tile_pool(name="xkq", bufs=1) as xkqp,
            tc.tile_pool(name="xv", bufs=2) as xvp,
            tc.tile_pool(name="ebA", bufs=EBA_BUFS) as ebAp,
            tc.tile_pool(name="ebB", bufs=EBB_BUFS) as ebBp,
            tc.tile_pool(name="pts", bufs=2) as ptsp,
            tc.tile_pool(name="cc", bufs=2) as ccp,
            tc.tile_pool(name="psp", bufs=3, space="PSUM") as psp,
            tc.tile_pool(name="post", bufs=1, space="PSUM") as post,
        ):
            trimask_sb = persist.tile([P, P], fp16, tag="trimask")
            ident_sb = persist.tile([P, P], fp16, tag="ident")
            nc.sync.dma_start(out=trimask_sb, in_=trimask[:])
            nc.sync.dma_start(out=ident_sb, in_=ident[:])

            wo_sb = persist.tile([P, MPAIRS, D], fp16, tag="wo")
            nc.sync.dma_start(out=wo_sb, in_=wo[:].rearrange("(m p) n -> p m n", p=P))
            wq_sb = persist.tile([P, KO, HD], fp16, tag="wq")
            wk_sb = persist.tile([P, KO, HD], fp16, tag="wk")
            wv_sb = persist.tile([P, KO, HD], fp16, tag="wv")
            nc.sync.dma_start(out=wq_sb, in_=wq[:].rearrange("(ko p) n -> p ko n", p=P))
            nc.sync.dma_start(out=wk_sb, in_=wk[:].rearrange("(ko p) n -> p ko n", p=P))
            nc.sync.dma_start(out=wv_sb, in_=wv[:].rearrange("(ko p) n -> p ko n", p=P))

            # persistent activations
            qt = persist.tile([P, MPAIRS, S], fp16, tag="qt")   # rows = hd % 128
            kt = persist.tile([P, MPAIRS, S], fp16, tag="kt")
            vv = persist.tile([P, SBLKS, HD], fp16, tag="vv")   # [t%128, t//128, hd]

            xq_r = xtq[:].rearrange("(ko p) s -> p ko s", p=P)
            xk_r = xtk[:].rearrange("(ko p) s -> p ko s", p=P)
            xv_r = xtv[:].rearrange("(ko p) s -> p ko s", p=P)

            Exp = mybir.ActivationFunctionType.Exp
            X = mybir.AxisListType.X
            state = {}

            xc_tiles = {}

            def emit_kq_load(nch):
                """DMA one 512-col chunk of X_k / X_q."""
                for src_r, tg in ((xk_r, "xk"), (xq_r, "xq")):
                    xc = xkqp.tile([P, KO, CH], fp16, tag=tg,
                                   name=f"{tg}{nch}")
                    xc_tiles[(tg, nch)] = xc
                    nc.sync.dma_start(
                        out=xc, in_=src_r[:, :, nch * CH : (nch + 1) * CH]
                    )

            def emit_kq_group(nch, proj, m):
                """One m-group of the k or q projection for chunk nch."""
                wsb, dst, tg = ((wk_sb, kt, "xk") if proj == "k"
                                else (wq_sb, qt, "xq"))
                xc = xc_tiles[(tg, nch)]
                ps = psp.tile([P, 2 * CH], f32, tag="ps",
                              name=f"{tg}p{nch}_{m}")
                for ko in range(KO):
                    nc.tensor.matmul(
                        ps[:, :CH],
                        lhsT=wsb[:, ko, m * P : (m + 1) * P],
                        rhs=xc[:, ko, :],
                        start=(ko == 0),
                        stop=(ko == KO - 1),
                    )
                nc.scalar.copy(
                    out=dst[:, m, nch * CH : (nch + 1) * CH],
                    in_=ps[:, :CH],
                )

            def emit_v_proj(t2):
                """Project X_v group t2 (two 128-col t-blocks) -> vv."""
                xc = xvp.tile([P, KO, 2 * P], fp16, tag="xv", name=f"xv{t2}")
                nc.sync.dma_start(
                    out=xc, in_=xv_r[:, :, t2 * 2 * P : (t2 + 1) * 2 * P]
                )
                ps = psp.tile([P, 2 * CH], f32, tag="ps", name=f"vp{t2}")
                for half in range(2):
                    for ko in range(KO):
                        nc.tensor.matmul(
                            ps[:, half * CH : (half + 1) * CH],
                            lhsT=xc[:, ko, half * P : (half + 1) * P],
                            rhs=wv_sb[:, ko, :],
                            start=(ko == 0),
                            stop=(ko == KO - 1),
                        )
                eng = nc.vector.tensor_copy if t2 % 2 == 0 else nc.scalar.copy
                eng(
                    out=vv[:, t2 * 2 : t2 * 2 + 2, :].rearrange("p a b -> p (a b)"),
                    in_=ps,
                )

            def emit_chains(i):
                c, r = i // 4, i % 4
                width = (i + 1) * P
                late = c >= 2
                wA = min(width, 2 * CH)
                wB = width - wA
                st = {"late": late, "wA": wA, "wB": wB, "pts": [], "ebuf": []}
                st["den"] = stats.tile([P, HLOC], f32, tag="den", name=f"den{i}")
                st["negmA"] = stats.tile([P, HLOC], f32, tag="negmA",
                                         name=f"negmA{i}")
                if late:
                    for t in ("negmB", "negm", "accA", "accB", "alpha"):
                        st[t] = stats.tile([P, HLOC], f32, tag=t, name=f"{t}{i}")
                for h in range(HLOC):
                    m, z = h // 2, h % 2
                    qts = qt[z * DK : (z + 1) * DK, m, i * P : (i + 1) * P]
                    tA = psp.tile([P, 2 * CH], f32, tag="ps", name=f"sA{i}_{h}")
                    for cc in range(min(c, 1) + 1):
                        w = CH if cc < c else (r + 1) * P
                        nc.tensor.matmul(
                            tA[:, cc * CH : cc * CH + w],
                            lhsT=qts,
                            rhs=kt[z * DK : (z + 1) * DK, m, cc * CH : cc * CH + w],
                            start=True,
                            stop=(cc != c),
                        )
                    if not late:
                        nc.tensor.matmul(
                            tA[:, c * CH + r * P : c * CH + (r + 1) * P],
                            lhsT=ident_sb, rhs=trimask_sb,
                            start=False, stop=True,
                        )
                    nc.vector.reduce_max(
                        st["negmA"][:, h : h + 1], tA[:, :wA], axis=X,
                        negate=True,
                    )
                    ebuf = ebAp.tile([P, 2 * CH], fp16, tag="eb",
                                     name=f"eb{i}_{h}")
                    st["ebuf"].append(ebuf)
                    pt = ptsp.tile([P, SBLKS, P], fp16, tag=f"pt{h}",
                                   name=f"pt{i}_{h}")
                    st["pts"].append(pt)
                    nc.scalar.activation(
                        out=ebuf[:, :wA], in_=tA[:, :wA], func=Exp,
                        bias=st["negmA"][:, h : h + 1], scale=1.0,
                        accum_out=(st["accA"] if late else st["den"])[:, h : h + 1],
                    )
                    if not late:
                        nc.sync.dma_start(
                            out=pt[:, 0 : i + 1, :], in_=ebuf[:, 0:width],
                            transpose=True,
                        )
                        continue
                    tB = psp.tile([P, 2 * CH], f32, tag="ps", name=f"sB{i}_{h}")
                    for cc in range(2, c + 1):
                        w = CH if cc < c else (r + 1) * P
                        nc.tensor.matmul(
                            tB[:, (cc - 2) * CH : (cc - 2) * CH + w],
                            lhsT=qts,
                            rhs=kt[z * DK : (z + 1) * DK, m, cc * CH : cc * CH + w],
                            start=True,
                            stop=(cc != c),
                        )
                    nc.tensor.matmul(
                        tB[:, (c - 2) * CH + r * P : (c - 2) * CH + (r + 1) * P],
                        lhsT=ident_sb, rhs=trimask_sb,
                        start=False, stop=True,
                    )
                    nc.vector.reduce_max(
                        st["negmB"][:, h : h + 1], tB[:, :wB], axis=X,
                        negate=True,
                    )
                    # negm = -max(mA, mB) = min(negmA, negmB)
                    nc.vector.tensor_tensor(
                        out=st["negm"][:, h : h + 1],
                        in0=st["negmA"][:, h : h + 1],
                        in1=st["negmB"][:, h : h + 1], op=mybir.AluOpType.min,
                    )
                    ebB = ebBp.tile([P, 2 * CH], fp16, tag="ebB",
                                    name=f"ebB{i}_{h}")
                    nc.scalar.activation(
                        out=ebB[:, :wB], in_=tB[:, :wB], func=Exp,
                        bias=st["negm"][:, h : h + 1], scale=1.0,
                        accum_out=st["accB"][:, h : h + 1],
                    )
                    # exact part B transposed now; part A in the tail
                    nc.sync.dma_start(
                        out=pt[:, 8 : i + 1, :], in_=ebB[:, :wB],
                        transpose=True,
                    )
                state[i] = st

            def emit_tail(i):
                st = state.pop(i)
                invden = stats.tile([P, HLOC], f32, tag="invden",
                                    name=f"invden{i}")
                if st["late"]:
                    # alpha = exp(mA - m) = exp(negm - negmA), batched [P, 8]
                    dmx = stats.tile([P, HLOC], f32, tag="dmx", name=f"dmx{i}")
                    nc.vector.tensor_tensor(
                        out=dmx, in0=st["negm"], in1=st["negmA"],
                        op=mybir.AluOpType.subtract,
                    )
                    nc.scalar.activation(out=st["alpha"], in_=dmx, func=Exp,
                                         bias=0.0, scale=1.0)
                    # den = alpha * accA + accB
                    nc.vector.tensor_tensor(out=st["accA"], in0=st["accA"],
                                            in1=st["alpha"],
                                            op=mybir.AluOpType.mult)
                    nc.vector.tensor_tensor(out=st["den"], in0=st["accA"],
                                            in1=st["accB"],
                                            op=mybir.AluOpType.add)
                    for h in range(HLOC):
                        nc.vector.tensor_scalar(
                            out=st["ebuf"][h][:, : st["wA"]],
                            in0=st["ebuf"][h][:, : st["wA"]],
                            scalar1=st["alpha"][:, h : h + 1],
                            scalar2=None,
                            op0=mybir.AluOpType.mult,
                        )
                        nc.sync.dma_start(
                            out=st["pts"][h][:, 0:8, :],
                            in_=st["ebuf"][h][:, : st["wA"]],
                            transpose=True,
                        )
                nc.vector.reciprocal(invden, st["den"])

                # PV: po[s, h*64:(h+1)*64] = sum_j E^T_j.T @ v_j
                po = post.tile([P, HD], f32, tag="po", name=f"po{i}")
                for h in range(HLOC):
                    for j in range(i + 1):
                        nc.tensor.matmul(
                            po[:, h * DK : (h + 1) * DK],
                            lhsT=st["pts"][h][:, j, :],
                            rhs=vv[:, j, h * DK : (h + 1) * DK],
                            start=(j == 0),
                            stop=(j == i),
                        )
                # normalize + copy to sbuf in one DVE op
                conc = ccp.tile([P, HD], fp16, tag="conc", name=f"conc{i}")
                nc.vector.tensor_tensor(
                    out=conc[:, :].rearrange("p (h k) -> p h k", h=HLOC),
                    in0=po[:, :].rearrange("p (h k) -> p h k", h=HLOC),
                    in1=invden[:, :, None].broadcast_to((P, HLOC, DK)),
                    op=mybir.AluOpType.mult,
                )
                # conc[s, hd] -> concT[hd%128, m, s-block]
                concT = ccp.tile([P, MPAIRS, P], fp16, tag="concT",
                                 name=f"concT{i}")
                nc.sync.dma_start(out=concT, in_=conc[:, :], transpose=True)
                # output projection for this s-block
                ysb = ccp.tile([P, D], fp16, tag="ysb", name=f"ysb{i}")
                for nch in range(2):
                    ypt = post.tile([P, CH], f32, tag="yp", name=f"yp{i}_{nch}")
                    for m in range(MPAIRS):
                        nc.tensor.matmul(
                            ypt,
                            lhsT=concT[:, m, :],
                            rhs=wo_sb[:, m, nch * CH : (nch + 1) * CH],
                            start=(m == 0),
                            stop=(m == MPAIRS - 1),
                        )
                    nc.scalar.copy(
                        out=ysb[:, nch * CH : (nch + 1) * CH], in_=ypt
                    )
                nc.sync.dma_start(out=y[:][i * P : (i + 1) * P, :], in_=ysb)

            # Block processing order: 1..15 then 0 so the final (drain) tail
            # is the smallest block. Projection chunk work is spread across
            # steps: chunk 0 fully at step 0; chunk nch>=1 split over the
            # three steps before the first block that needs it completes.
            blk_order = list(BLK_ORDER)
            proj_work = {s: [] for s in range(SBLKS)}
            proj_work[0] = [("load", 0)] + [
                ("grp", 0, p, m) for p in ("k", "q") for m in range(MPAIRS)
            ]
            for nch in range(1, 4):
                groups = [("grp", nch, p, m) for p in ("k", "q")
                          for m in range(MPAIRS)]
                steps = PROJ_STEPS[nch]
                proj_work[min(min(steps), 4 * (nch - 1))].append(("load", nch))
                for gi, g in enumerate(groups):
                    proj_work[steps[gi]].append(g)

            for s in range(SBLKS + 1):
                if s < SBLKS:
                    for w in proj_work[s]:
                        if w[0] == "load":
                            emit_kq_load(w[1])
                        else:
                            emit_kq_group(w[1], w[2], w[3])
                    if s < 8:
                        emit_v_proj(s)
                    emit_chains(blk_order[s])
                if s >= 1:
                    emit_tail(blk_order[s - 1])

    nc.finalize()
    return nc


def _prep_inputs(Q, K, V, Wq, Wk, Wv, Wo):
    """Host-side shard + layout prep. Returns list of 8 in_maps."""
    rt8 = math.sqrt(math.sqrt(64.0))  # sqrt(8): scale split over q and k
    tri = np.where(
        np.arange(P)[None, :] <= np.arange(P)[:, None], 0.0, NEG
    ).astype(np.float16)
    ident = np.eye(P, dtype=np.float16)
    in_maps = []
    for c in range(8):
        b, g = c // 2, c % 2
        heads = slice(g * HLOC, (g + 1) * HLOC)
        wq_p = (Wq[heads] * rt8).transpose(1, 0, 2).reshape(D, HD)
        wk_p = (Wk[heads] * rt8).transpose(1, 0, 2).reshape(D, HD)
        wv_p = Wv[heads].transpose(1, 0, 2).reshape(D, HD)
        wo_p = Wo[:, g * HD : (g + 1) * HD].T  # [HD, D]
        in_maps.append({
            "xtq": np.ascontiguousarray(Q[b].T).astype(np.float16),
            "xtk": np.ascontiguousarray(K[b].T).astype(np.float16),
            "xtv": np.ascontiguousarray(V[b].T).astype(np.float16),
            "wq": np.ascontiguousarray(wq_p).astype(np.float16),
            "wk": np.ascontiguousarray(wk_p).astype(np.float16),
            "wv": np.ascontiguousarray(wv_p).astype(np.float16),
            "wo": np.ascontiguousarray(wo_p).astype(np.float16),
            "trimask": tri,
            "ident": ident,
        })
    return in_maps


_NC = []


def kernel(Q, K, V, mask, Wq, Wk, Wv, Wo, bo, _trace=False):
    from concourse.bass_utils import run_bass_kernel_spmd

    Q, K, V = np.asarray(Q), np.asarray(K), np.asarray(V)
    Wq, Wk, Wv = np.asarray(Wq), np.asarray(Wk), np.asarray(Wv)
    Wo, bo = np.asarray(Wo), np.asarray(bo)

    if not _NC:
        _NC.append(build())
    nc = _NC[0]
    in_maps = _prep_inputs(Q, K, V, Wq, Wk, Wv, Wo)
    res = run_bass_kernel_spmd(nc, in_maps, core_ids=list(range(8)), trace=_trace)
    ys = [r["y"].astype(np.float32) for r in res.results]
    out = np.stack([ys[2 * b] + ys[2 * b + 1] for b in range(B)])
    out = out + bo[None, None, :].astype(np.float32)
    if _trace:
        kernel._last = res
    return out.astype(np.float32)


# revision 49
# speedup vs baseline: 1.0580x; 1.0036x over previous
"""Multi-head attention (B=4, S=2048, D=1024, H=16, causal) on 8 trn2 cores.

Sharding: data-parallel over batch (4) x tensor-parallel over head groups (2).
Core c handles batch b=c//2, heads g=c%2 (8 heads each). Each core computes
its partial output projection; host sums the two partials per batch and adds
the bias.

Per-core pipeline (all matmul inputs fp16, fp32 accumulation). The q/k/v
projections are interleaved with the attention blocks (k/q emitted per
512-col chunk right before the four s-blocks that first need them, v per
256-col group during the early blocks) so the projection's PE work overlaps
the attention's DVE/ACT work. Attention per s-block i and local head h uses
an online two-part softmax so score psum tiles are freed early (psum is the
concurrency limiter), software-pipelined one block deep (chains of block i
are emitted before the tail of block i-1 so the PE stream never head-of-line
blocks on the tail's transposes):
  chains(i): scores via K=64 matmuls (causal mask added on the PE via an
    identity-weight matmul of a constant triangular NEG tile); row-max per
    part (DVE); exp(bias=-part max) with accum_out denominator (ACT) ->
    unnormalized E fp16; exact part B transposed to E^T immediately.
  tail(i): part A correction alpha = exp(mA - m) applied to E_A on the
    otherwise-idle gpsimd engine; den = alpha*accA + accB; part A
    transposed; PV in out[s,dk] orientation (po += E^T_j.T @ v_j);
    deferred normalization po * (1/den) fused into the psum->sbuf copy
    (DVE); concat -> concT via DMA transpose; y = concT.T @ Wo -> fp16.
"""

import math

import numpy as np

B, S, D, H = 4, 2048, 1024, 16
DK = 64
HLOC = 8          # heads per core
HD = HLOC * DK    # 512 local concat dims
P = 128
SBLKS = S // P    # 16
CH = 512          # score chunk width
KO = D // P       # 8 contraction tiles for projections
MPAIRS = 4        # head pairs per core
NEG = -30000.0

# schedule/buffer knobs (module-level so they can be tuned)
BLK_ORDER = [2, 3, 4, 5, 6, 7, 8, 9, 10, 11, 15, 14, 13, 12, 0, 1]
EBA_BUFS = 11
EBB_BUFS = 4
# per-chunk list of steps at which the 8 (k,q) m-groups are emitted
PROJ_STEPS = {1: [0, 0, 0, 1, 1, 1, 1, 1],
              2: [2, 2, 3, 3, 4, 4, 5, 5],
              3: [5, 6, 6, 7, 7, 8, 8, 9]}


def build():
    import concourse.bass as bass
    import concourse.mybir as mybir
    import concourse.tile as tile
    from concourse import bacc

    fp16 = mybir.dt.float16
    f32 = mybir.dt.float32

    nc = bacc.Bacc()

    xtq = nc.dram_tensor("xtq", [D, S], fp16, kind="ExternalInput")
    xtk = nc.dram_tensor("xtk", [D, S], fp16, kind="ExternalInput")
    xtv = nc.dram_tensor("xtv", [D, S], fp16, kind="ExternalInput")
    wq = nc.dram_tensor("wq", [D, HD], fp16, kind="ExternalInput")
    wk = nc.dram_tensor("wk", [D, HD], fp16, kind="ExternalInput")
    wv = nc.dram_tensor("wv", [D, HD], fp16, kind="ExternalInput")
    wo = nc.dram_tensor("wo", [HD, D], fp16, kind="ExternalInput")
    trimask = nc.dram_tensor("trimask", [P, P], fp16, kind="ExternalInput")
    ident = nc.dram_tensor("ident", [P, P], fp16, kind="ExternalInput")
    y = nc.dram_tensor("y", [S, D], fp16, kind="ExternalOutput")

    with tile.TileContext(nc) as tc:
        with (
            tc.tile_pool(name="persist", bufs=1) as persist,
            tc.tile_pool(name="stats", bufs=2) as stats,
            tc.tile_pool(name="xkq", bufs=1) as xkqp,
            tc.tile_pool(name="xv", bufs=2) as xvp,
            tc.tile_pool(name="ebA", bufs=EBA_BUFS) as ebAp,
            tc.tile_pool(name="ebB", bufs=EBB_BUFS) as ebBp,
            tc.tile_pool(name="pts", bufs=2) as ptsp,
            tc.tile_pool(name="cc", bufs=2) as ccp,
            tc.tile_pool(name="psp", bufs=3, space="PSUM") as psp,
            tc.tile_pool(name="post", bufs=1, space="PSUM") as post,
        ):
            trimask_sb = persist.tile([P, P], fp16, tag="trimask")
            ident_sb = persist.tile([P, P], fp16, tag="ident")
            nc.sync.dma_start(out=trimask_sb, in_=trimask[:])
            nc.sync.dma_start(out=ident_sb, in_=ident[:])

            wo_sb = persist.tile([P, MPAIRS, D], fp16, tag="wo")
            nc.sync.dma_start(out=wo_sb, in_=wo[:].rearrange("(m p) n -> p m n", p=P))
            wq_sb = persist.tile([P, KO, HD], fp16, tag="wq")
            wk_sb = persist.tile([P, KO, HD], fp16, tag="wk")
            wv_sb = persist.tile([P, KO, HD], fp16, tag="wv")
            nc.sync.dma_start(out=wq_sb, in_=wq[:].rearrange("(ko p) n -> p ko n", p=P))
            nc.sync.dma_start(out=wk_sb, in_=wk[:].rearrange("(ko p) n -> p ko n", p=P))
            nc.sync.dma_start(out=wv_sb, in_=wv[:].rearrange("(ko p) n -> p ko n", p=P))

            # persistent activations
            qt = persist.tile([P, MPAIRS, S], fp16, tag="qt")   # rows = hd % 128
            kt = persist.tile([P, MPAIRS, S], fp16, tag="kt")
            vv = persist.tile([P, SBLKS, HD], fp16, tag="vv")   # [t%128, t//128, hd]

            xq_r = xtq[:].rearrange("(ko p) s -> p ko s", p=P)
            xk_r = xtk[:].rearrange("(ko p) s -> p ko s", p=P)
            xv_r = xtv[:].rearrange("(ko p) s -> p ko s", p=P)

            Exp = mybir.ActivationFunctionType.Exp
            X = mybir.AxisListType.X
            state = {}

            xc_tiles = {}

            def emit_kq_load(nch):
                """DMA one 512-col chunk of X_k / X_q."""
                for src_r, tg in ((xk_r, "xk"), (xq_r, "xq")):
                    xc = xkqp.tile([P, KO, CH], fp16, tag=tg,
                                   name=f"{tg}{nch}")
                    xc_tiles[(tg, nch)] = xc
                    nc.sync.dma_start(
                        out=xc, in_=src_r[:, :, nch * CH : (nch + 1) * CH]
                    )

            def emit_kq_group(nch, proj, m):
                """One m-group of the k or q projection for chunk nch."""
                wsb, dst, tg = ((wk_sb, kt, "xk") if proj == "k"
                                else (wq_sb, qt, "xq"))
                xc = xc_tiles[(tg, nch)]
                ps = psp.tile([P, 2 * CH], f32, tag="ps",
                              name=f"{tg}p{nch}_{m}")
                for ko in range(KO):
                    nc.tensor.matmul(
                        ps[:, :CH],
                        lhsT=wsb[:, ko, m * P : (m + 1) * P],
                        rhs=xc[:, ko, :],
                        start=(ko == 0),
                        stop=(ko == KO - 1),
                    )
                nc.scalar.copy(
                    out=dst[:, m, nch * CH : (nch + 1) * CH],
                    in_=ps[:, :CH],
                )

            def emit_v_proj(t2):
                """Project X_v group t2 (two 128-col t-blocks) -> vv."""
                xc = xvp.tile([P, KO, 2 * P], fp16, tag="xv", name=f"xv{t2}")
                nc.sync.dma_start(
                    out=xc, in_=xv_r[:, :, t2 * 2 * P : (t2 + 1) * 2 * P]
                )
                ps = psp.tile([P, 2 * CH], f32, tag="ps", name=f"vp{t2}")
                for half in range(2):
                    for ko in range(KO):
                        nc.tensor.matmul(
                            ps[:, half * CH : (half + 1) * CH],
                            lhsT=xc[:, ko, half * P : (half + 1) * P],
                            rhs=wv_sb[:, ko, :],
                            start=(ko == 0),
                            stop=(ko == KO - 1),
                        )
                eng = nc.scalar.copy
                eng(
                    out=vv[:, t2 * 2 : t2 * 2 + 2, :].rearrange("p a b -> p (a b)"),
                    in_=ps,
                )

            def emit_chains(i):
                c, r = i // 4, i % 4
                width = (i + 1) * P
                late = c >= 2
                wA = min(width, 2 * CH)
                wB = width - wA
                st = {"late": late, "wA": wA, "wB": wB, "pts": [], "ebuf": []}
                st["den"] = stats.tile([P, HLOC], f32, tag="den", name=f"den{i}")
                st["negmA"] = stats.tile([P, HLOC], f32, tag="negmA",
                                         name=f"negmA{i}")
                if late:
                    for t in ("negmB", "negm", "accA", "accB", "alpha"):
                        st[t] = stats.tile([P, HLOC], f32, tag=t, name=f"{t}{i}")
                for h in range(HLOC):
                    m, z = h // 2, h % 2
                    qts = qt[z * DK : (z + 1) * DK, m, i * P : (i + 1) * P]
                    tA = psp.tile([P, 2 * CH], f32, tag="ps", name=f"sA{i}_{h}")
                    for cc in range(min(c, 1) + 1):
                        w = CH if cc < c else (r + 1) * P
                        nc.tensor.matmul(
                            tA[:, cc * CH : cc * CH + w],
                            lhsT=qts,
                            rhs=kt[z * DK : (z + 1) * DK, m, cc * CH : cc * CH + w],
                            start=True,
                            stop=(cc != c),
                        )
                    if not late:
                        nc.tensor.matmul(
                            tA[:, c * CH + r * P : c * CH + (r + 1) * P],
                            lhsT=ident_sb, rhs=trimask_sb,
                            start=False, stop=True,
                        )
                    nc.vector.reduce_max(
                        st["negmA"][:, h : h + 1], tA[:, :wA], axis=X,
                        negate=True,
                    )
                    ebuf = ebAp.tile([P, 2 * CH], fp16, tag="eb",
                                     name=f"eb{i}_{h}")
                    st["ebuf"].append(ebuf)
                    pt = ptsp.tile([P, SBLKS, P], fp16, tag=f"pt{h}",
                                   name=f"pt{i}_{h}")
                    st["pts"].append(pt)
                    nc.scalar.activation(
                        out=ebuf[:, :wA], in_=tA[:, :wA], func=Exp,
                        bias=st["negmA"][:, h : h + 1], scale=1.0,
                        accum_out=(st["accA"] if late else st["den"])[:, h : h + 1],
                    )
                    if not late:
                        nc.sync.dma_start(
                            out=pt[:, 0 : i + 1, :], in_=ebuf[:, 0:width],
                            transpose=True,
                        )
                        continue
                    tB = psp.tile([P, 2 * CH], f32, tag="ps", name=f"sB{i}_{h}")
                    for cc in range(2, c + 1):
                        w = CH if cc < c else (r + 1) * P
                        nc.tensor.matmul(
                            tB[:, (cc - 2) * CH : (cc - 2) * CH + w],
                            lhsT=qts,
                            rhs=kt[z * DK : (z + 1) * DK, m, cc * CH : cc * CH + w],
                            start=True,
                            stop=(cc != c),
                        )
                    nc.tensor.matmul(
                        tB[:, (c - 2) * CH + r * P : (c - 2) * CH + (r + 1) * P],
                        lhsT=ident_sb, rhs=trimask_sb,
                        start=False, stop=True,
                    )
                    nc.vector.reduce_max(
                        st["negmB"][:, h : h + 1], tB[:, :wB], axis=X,
                        negate=True,
                    )
                    # negm = -max(mA, mB) = min(negmA, negmB)
                    nc.vector.tensor_tensor(
                        out=st["negm"][:, h : h + 1],
                        in0=st["negmA"][:, h : h + 1],
                        in1=st["negmB"][:, h : h + 1], op=mybir.AluOpType.min,
                    )
                    ebB = ebBp.tile([P, 2 * CH], fp16, tag="ebB",
                                    name=f"ebB{i}_{h}")
                    nc.scalar.activation(
                        out=ebB[:, :wB], in_=tB[:, :wB], func=Exp,
                        bias=st["negm"][:, h : h + 1], scale=1.0,
                        accum_out=st["accB"][:, h : h + 1],
                    )
                    # exact part B transposed now; part A in the tail
                    nc.sync.dma_start(
                        out=pt[:, 8 : i + 1, :], in_=ebB[:, :wB],
                        transpose=True,
                    )
                state[i] = st

            def emit_tail(i):
                st = state.pop(i)
                invden = stats.tile([P, HLOC], f32, tag="invden",
                                    name=f"invden{i}")
                if st["late"]:
                    # alpha = exp(mA - m) = exp(negm - negmA), batched [P, 8]
                    dmx = stats.tile([P, HLOC], f32, tag="dmx", name=f"dmx{i}")
                    nc.vector.tensor_tensor(
                        out=dmx, in0=st["negm"], in1=st["negmA"],
                        op=mybir.AluOpType.subtract,
                    )
                    nc.scalar.activation(out=st["alpha"], in_=dmx, func=Exp,
                                         bias=0.0, scale=1.0)
                    # den = alpha * accA + accB
                    nc.vector.tensor_tensor(out=st["accA"], in0=st["accA"],
                                            in1=st["alpha"],
                                            op=mybir.AluOpType.mult)
                    nc.vector.tensor_tensor(out=st["den"], in0=st["accA"],
                                            in1=st["accB"],
                                            op=mybir.AluOpType.add)
                    for h in range(HLOC):
                        nc.vector.tensor_scalar(
                            out=st["ebuf"][h][:, : st["wA"]],
                            in0=st["ebuf"][h][:, : st["wA"]],
                            scalar1=st["alpha"][:, h : h + 1],
                            scalar2=None,
                            op0=mybir.AluOpType.mult,
                        )
                        nc.sync.dma_start(
                            out=st["pts"][h][:, 0:8, :],
                            in_=st["ebuf"][h][:, : st["wA"]],
                            transpose=True,
                        )
                nc.vector.reciprocal(invden, st["den"])

                # PV: po[s, h*64:(h+1)*64] = sum_j E^T_j.T @ v_j
                po = post.tile([P, HD], f32, tag="po", name=f"po{i}")
                for h in range(HLOC):
                    for j in range(i + 1):
                        nc.tensor.matmul(
                            po[:, h * DK : (h + 1) * DK],
                            lhsT=st["pts"][h][:, j, :],
                            rhs=vv[:, j, h * DK : (h + 1) * DK],
                            start=(j == 0),
                            stop=(j == i),
                        )
                # normalize + copy to sbuf in one DVE op
                conc = ccp.tile([P, HD], fp16, tag="conc", name=f"conc{i}")
                nc.vector.tensor_tensor(
                    out=conc[:, :].rearrange("p (h k) -> p h k", h=HLOC),
                    in0=po[:, :].rearrange("p (h k) -> p h k", h=HLOC),
                    in1=invden[:, :, None].broadcast_to((P, HLOC, DK)),
                    op=mybir.AluOpType.mult,
                )
                # conc[s, hd] -> concT[hd%128, m, s-block]
                concT = ccp.tile([P, MPAIRS, P], fp16, tag="concT",
                                 name=f"concT{i}")
                nc.sync.dma_start(out=concT, in_=conc[:, :], transpose=True)
                # output projection for this s-block
                ysb = ccp.tile([P, D], fp16, tag="ysb", name=f"ysb{i}")
                for nch in range(2):
                    ypt = post.tile([P, CH], f32, tag="yp", name=f"yp{i}_{nch}")
                    for m in range(MPAIRS):
                        nc.tensor.matmul(
                            ypt,
                            lhsT=concT[:, m, :],
                            rhs=wo_sb[:, m, nch * CH : (nch + 1) * CH],
                            start=(m == 0),
                            stop=(m == MPAIRS - 1),
                        )
                    nc.scalar.copy(
                        out=ysb[:, nch * CH : (nch + 1) * CH], in_=ypt
                    )
                nc.sync.dma_start(out=y[:][i * P : (i + 1) * P, :], in_=ysb)

            # Block processing order: 1..15 then 0 so the final (drain) tail
            # is the smallest block. Projection chunk work is spread across
            # steps: chunk 0 fully at step 0; chunk nch>=1 split over the
            # three steps before the first block that needs it completes.
            blk_order = list(BLK_ORDER)
            proj_work = {s: [] for s in range(SBLKS)}
            proj_work[0] = [("load", 0)] + [
                ("grp", 0, p, m) for p in ("k", "q") for m in range(MPAIRS)
            ]
            for nch in range(1, 4):
                groups = [("grp", nch, p, m) for p in ("k", "q")
                          for m in range(MPAIRS)]
                steps = PROJ_STEPS[nch]
                proj_work[min(min(steps), 4 * (nch - 1))].append(("load", nch))
                for gi, g in enumerate(groups):
                    proj_work[steps[gi]].append(g)

            for s in range(SBLKS + 1):
                if s < SBLKS:
                    for w in proj_work[s]:
                        if w[0] == "load":
                            emit_kq_load(w[1])
                        else:
                            emit_kq_group(w[1], w[2], w[3])
                    if s < 8:
                        emit_v_proj(s)
                    emit_chains(blk_order[s])
                if s >= 1:
                    emit_tail(blk_order[s - 1])

    nc.finalize()
    return nc


def _prep_inputs(Q, K, V, Wq, Wk, Wv, Wo):
    """Host-side shard + layout prep. Returns list of 8 in_maps."""
    rt8 = math.sqrt(math.sqrt(64.0))  # sqrt(8): scale split over q and k
    tri = np.where(
        np.arange(P)[None, :] <= np.arange(P)[:, None], 0.0, NEG
    ).astype(np.float16)
    ident = np.eye(P, dtype=np.float16)
    in_maps = []
    for c in range(8):
        b, g = c // 2, c % 2
        heads = slice(g * HLOC, (g + 1) * HLOC)
        wq_p = (Wq[heads] * rt8).transpose(1, 0, 2).reshape(D, HD)
        wk_p = (Wk[heads] * rt8).transpose(1, 0, 2).reshape(D, HD)
        wv_p = Wv[heads].transpose(1, 0, 2).reshape(D, HD)
        wo_p = Wo[:, g * HD : (g + 1) * HD].T  # [HD, D]
        in_maps.append({
            "xtq": np.ascontiguousarray(Q[b].T).astype(np.float16),
            "xtk": np.ascontiguousarray(K[b].T).astype(np.float16),
            "xtv": np.ascontiguousarray(V[b].T).astype(np.float16),
            "wq": np.ascontiguousarray(wq_p).astype(np.float16),
            "wk": np.ascontiguousarray(wk_p).astype(np.float16),
            "wv": np.ascontiguousarray(wv_p).astype(np.float16),
            "wo": np.ascontiguousarray(wo_p).astype(np.float16),
            "trimask": tri,
            "ident": ident,
        })
    return in_maps


_NC = []


def kernel(Q, K, V, mask, Wq, Wk, Wv, Wo, bo, _trace=False):
    from concourse.bass_utils import run_bass_kernel_spmd

    Q, K, V = np.asarray(Q), np.asarray(K), np.asarray(V)
    Wq, Wk, Wv = np.asarray(Wq), np.asarray(Wk), np.asarray(Wv)
    Wo, bo = np.asarray(Wo), np.asarray(bo)

    if not _NC:
        _NC.append(build())
    nc = _NC[0]
    in_maps = _prep_inputs(Q, K, V, Wq, Wk, Wv, Wo)
    res = run_bass_kernel_spmd(nc, in_maps, core_ids=list(range(8)), trace=_trace)
    ys = [r["y"].astype(np.float32) for r in res.results]
    out = np.stack([ys[2 * b] + ys[2 * b + 1] for b in range(B)])
    out = out + bo[None, None, :].astype(np.float32)
    if _trace:
        kernel._last = res
    return out.astype(np.float32)


# revision 53
# speedup vs baseline: 1.0619x; 1.0037x over previous
"""Multi-head attention (B=4, S=2048, D=1024, H=16, causal) on 8 trn2 cores.

Sharding: data-parallel over batch (4) x tensor-parallel over head groups (2).
Core c handles batch b=c//2, heads g=c%2 (8 heads each). Each core computes
its partial output projection; host sums the two partials per batch and adds
the bias.

Per-core pipeline (all matmul inputs fp16, fp32 accumulation). The q/k/v
projections are interleaved with the attention blocks (k/q emitted per
512-col chunk right before the four s-blocks that first need them, v per
256-col group during the early blocks) so the projection's PE work overlaps
the attention's DVE/ACT work. Attention per s-block i and local head h uses
an online two-part softmax so score psum tiles are freed early (psum is the
concurrency limiter), software-pipelined one block deep (chains of block i
are emitted before the tail of block i-1 so the PE stream never head-of-line
blocks on the tail's transposes):
  chains(i): scores via K=64 matmuls (causal mask added on the PE via an
    identity-weight matmul of a constant triangular NEG tile); row-max per
    part (DVE); exp(bias=-part max) with accum_out denominator (ACT) ->
    unnormalized E fp16; exact part B transposed to E^T immediately.
  tail(i): part A correction alpha = exp(mA - m) applied to E_A on the
    otherwise-idle gpsimd engine; den = alpha*accA + accB; part A
    transposed; PV in out[s,dk] orientation (po += E^T_j.T @ v_j);
    deferred normalization po * (1/den) fused into the psum->sbuf copy
    (DVE); concat -> concT via DMA transpose; y = concT.T @ Wo -> fp16.
"""

import math

import numpy as np

B, S, D, H = 4, 2048, 1024, 16
DK = 64
HLOC = 8          # heads per core
HD = HLOC * DK    # 512 local concat dims
P = 128
SBLKS = S // P    # 16
CH = 512          # score chunk width
KO = D // P       # 8 contraction tiles for projections
MPAIRS = 4        # head pairs per core
NEG = -30000.0

# schedule/buffer knobs (module-level so they can be tuned)
BLK_ORDER = [2, 3, 4, 5, 6, 7, 8, 9, 10, 11, 15, 14, 13, 12, 0, 1]
EBA_BUFS = 11
EBB_BUFS = 4
# per-chunk list of steps at which the 8 (k,q) m-groups are emitted
PROJ_STEPS = {1: [0, 0, 0, 1, 1, 1, 1, 1],
              2: [2, 2, 3, 3, 4, 4, 5, 5],
              3: [5, 6, 6, 7, 7, 8, 8, 9]}
# valid iff V_STEPS[t2] <= 2*t2-1 (first reader: tail of block 2*t2)
V_STEPS = [1, 1, 2, 3, 4, 5, 6, 7]


def build():
    import concourse.bass as bass
    import concourse.mybir as mybir
    import concourse.tile as tile
    from concourse import bacc

    fp16 = mybir.dt.float16
    f32 = mybir.dt.float32

    nc = bacc.Bacc()

    xtq = nc.dram_tensor("xtq", [D, S], fp16, kind="ExternalInput")
    xtk = nc.dram_tensor("xtk", [D, S], fp16, kind="ExternalInput")
    xtv = nc.dram_tensor("xtv", [D, S], fp16, kind="ExternalInput")
    wq = nc.dram_tensor("wq", [D, HD], fp16, kind="ExternalInput")
    wk = nc.dram_tensor("wk", [D, HD], fp16, kind="ExternalInput")
    wv = nc.dram_tensor("wv", [D, HD], fp16, kind="ExternalInput")
    wo = nc.dram_tensor("wo", [HD, D], fp16, kind="ExternalInput")
    trimask = nc.dram_tensor("trimask", [P, P], fp16, kind="ExternalInput")
    ident = nc.dram_tensor("ident", [P, P], fp16, kind="ExternalInput")
    y = nc.dram_tensor("y", [S, D], fp16, kind="ExternalOutput")

    with tile.TileContext(nc) as tc:
        with (
            tc.tile_pool(name="persist", bufs=1) as persist,
            tc.tile_pool(name="stats", bufs=2) as stats,
            tc.tile_pool(name="xkq", bufs=1) as xkqp,
            tc.tile_pool(name="xv", bufs=2) as xvp,
            tc.tile_pool(name="ebA", bufs=EBA_BUFS) as ebAp,
            tc.tile_pool(name="ebB", bufs=EBB_BUFS) as ebBp,
            tc.tile_pool(name="pts", bufs=2) as ptsp,
            tc.tile_pool(name="cc", bufs=2) as ccp,
            tc.tile_pool(name="psp", bufs=3, space="PSUM") as psp,
            tc.tile_pool(name="post", bufs=1, space="PSUM") as post,
        ):
            trimask_sb = persist.tile([P, P], fp16, tag="trimask")
            ident_sb = persist.tile([P, P], fp16, tag="ident")
            nc.sync.dma_start(out=trimask_sb, in_=trimask[:])
            nc.sync.dma_start(out=ident_sb, in_=ident[:])

            wo_sb = persist.tile([P, MPAIRS, D], fp16, tag="wo")
            nc.sync.dma_start(out=wo_sb, in_=wo[:].rearrange("(m p) n -> p m n", p=P))
            wq_sb = persist.tile([P, KO, HD], fp16, tag="wq")
            wk_sb = persist.tile([P, KO, HD], fp16, tag="wk")
            wv_sb = persist.tile([P, KO, HD], fp16, tag="wv")
            nc.sync.dma_start(out=wq_sb, in_=wq[:].rearrange("(ko p) n -> p ko n", p=P))
            nc.sync.dma_start(out=wk_sb, in_=wk[:].rearrange("(ko p) n -> p ko n", p=P))
            nc.sync.dma_start(out=wv_sb, in_=wv[:].rearrange("(ko p) n -> p ko n", p=P))

            # persistent activations
            qt = persist.tile([P, MPAIRS, S], fp16, tag="qt")   # rows = hd % 128
            kt = persist.tile([P, MPAIRS, S], fp16, tag="kt")
            vv = persist.tile([P, SBLKS, HD], fp16, tag="vv")   # [t%128, t//128, hd]

            xq_r = xtq[:].rearrange("(ko p) s -> p ko s", p=P)
            xk_r = xtk[:].rearrange("(ko p) s -> p ko s", p=P)
            xv_r = xtv[:].rearrange("(ko p) s -> p ko s", p=P)

            Exp = mybir.ActivationFunctionType.Exp
            X = mybir.AxisListType.X
            state = {}

            xc_tiles = {}

            def emit_kq_load(nch):
                """DMA one 512-col chunk of X_k / X_q."""
                for src_r, tg in ((xk_r, "xk"), (xq_r, "xq")):
                    xc = xkqp.tile([P, KO, CH], fp16, tag=tg,
                                   name=f"{tg}{nch}")
                    xc_tiles[(tg, nch)] = xc
                    nc.sync.dma_start(
                        out=xc, in_=src_r[:, :, nch * CH : (nch + 1) * CH]
                    )

            def emit_kq_group(nch, proj, m):
                """One m-group of the k or q projection for chunk nch."""
                wsb, dst, tg = ((wk_sb, kt, "xk") if proj == "k"
                                else (wq_sb, qt, "xq"))
                xc = xc_tiles[(tg, nch)]
                ps = psp.tile([P, 2 * CH], f32, tag="ps",
                              name=f"{tg}p{nch}_{m}")
                for ko in range(KO):
                    nc.tensor.matmul(
                        ps[:, :CH],
                        lhsT=wsb[:, ko, m * P : (m + 1) * P],
                        rhs=xc[:, ko, :],
                        start=(ko == 0),
                        stop=(ko == KO - 1),
                    )
                nc.scalar.copy(
                    out=dst[:, m, nch * CH : (nch + 1) * CH],
                    in_=ps[:, :CH],
                )

            def emit_v_proj(t2):
                """Project X_v group t2 (two 128-col t-blocks) -> vv."""
                xc = xvp.tile([P, KO, 2 * P], fp16, tag="xv", name=f"xv{t2}")
                nc.sync.dma_start(
                    out=xc, in_=xv_r[:, :, t2 * 2 * P : (t2 + 1) * 2 * P]
                )
                ps = psp.tile([P, 2 * CH], f32, tag="ps", name=f"vp{t2}")
                for half in range(2):
                    for ko in range(KO):
                        nc.tensor.matmul(
                            ps[:, half * CH : (half + 1) * CH],
                            lhsT=xc[:, ko, half * P : (half + 1) * P],
                            rhs=wv_sb[:, ko, :],
                            start=(ko == 0),
                            stop=(ko == KO - 1),
                        )
                eng = nc.scalar.copy
                eng(
                    out=vv[:, t2 * 2 : t2 * 2 + 2, :].rearrange("p a b -> p (a b)"),
                    in_=ps,
                )

            def emit_chains(i):
                c, r = i // 4, i % 4
                width = (i + 1) * P
                late = c >= 2
                wA = min(width, 2 * CH)
                wB = width - wA
                st = {"late": late, "wA": wA, "wB": wB, "pts": [], "ebuf": []}
                st["den"] = stats.tile([P, HLOC], f32, tag="den", name=f"den{i}")
                st["negmA"] = stats.tile([P, HLOC], f32, tag="negmA",
                                         name=f"negmA{i}")
                if late:
                    for t in ("negmB", "negm", "accA", "accB", "alpha"):
                        st[t] = stats.tile([P, HLOC], f32, tag=t, name=f"{t}{i}")
                for h in range(HLOC):
                    m, z = h // 2, h % 2
                    qts = qt[z * DK : (z + 1) * DK, m, i * P : (i + 1) * P]
                    tA = psp.tile([P, 2 * CH], f32, tag="ps", name=f"sA{i}_{h}")
                    for cc in range(min(c, 1) + 1):
                        w = CH if cc < c else (r + 1) * P
                        nc.tensor.matmul(
                            tA[:, cc * CH : cc * CH + w],
                            lhsT=qts,
                            rhs=kt[z * DK : (z + 1) * DK, m, cc * CH : cc * CH + w],
                            start=True,
                            stop=(cc != c),
                        )
                    if not late:
                        nc.tensor.matmul(
                            tA[:, c * CH + r * P : c * CH + (r + 1) * P],
                            lhsT=ident_sb, rhs=trimask_sb,
                            start=False, stop=True,
                        )
                    nc.vector.reduce_max(
                        st["negmA"][:, h : h + 1], tA[:, :wA], axis=X,
                        negate=True,
                    )
                    ebuf = ebAp.tile([P, 2 * CH], fp16, tag="eb",
                                     name=f"eb{i}_{h}")
                    st["ebuf"].append(ebuf)
                    pt = ptsp.tile([P, SBLKS, P], fp16, tag=f"pt{h}",
                                   name=f"pt{i}_{h}")
                    st["pts"].append(pt)
                    nc.scalar.activation(
                        out=ebuf[:, :wA], in_=tA[:, :wA], func=Exp,
                        bias=st["negmA"][:, h : h + 1], scale=1.0,
                        accum_out=(st["accA"] if late else st["den"])[:, h : h + 1],
                    )
                    if not late:
                        nc.sync.dma_start(
                            out=pt[:, 0 : i + 1, :], in_=ebuf[:, 0:width],
                            transpose=True,
                        )
                        continue
                    tB = psp.tile([P, 2 * CH], f32, tag="ps", name=f"sB{i}_{h}")
                    for cc in range(2, c + 1):
                        w = CH if cc < c else (r + 1) * P
                        nc.tensor.matmul(
                            tB[:, (cc - 2) * CH : (cc - 2) * CH + w],
                            lhsT=qts,
                            rhs=kt[z * DK : (z + 1) * DK, m, cc * CH : cc * CH + w],
                            start=True,
                            stop=(cc != c),
                        )
                    nc.tensor.matmul(
                        tB[:, (c - 2) * CH + r * P : (c - 2) * CH + (r + 1) * P],
                        lhsT=ident_sb, rhs=trimask_sb,
                        start=False, stop=True,
                    )
                    nc.vector.reduce_max(
                        st["negmB"][:, h : h + 1], tB[:, :wB], axis=X,
                        negate=True,
                    )
                    # negm = -max(mA, mB) = min(negmA, negmB)
                    nc.vector.tensor_tensor(
                        out=st["negm"][:, h : h + 1],
                        in0=st["negmA"][:, h : h + 1],
                        in1=st["negmB"][:, h : h + 1], op=mybir.AluOpType.min,
                    )
                    ebB = ebBp.tile([P, 2 * CH], fp16, tag="ebB",
                                    name=f"ebB{i}_{h}")
                    nc.scalar.activation(
                        out=ebB[:, :wB], in_=tB[:, :wB], func=Exp,
                        bias=st["negm"][:, h : h + 1], scale=1.0,
                        accum_out=st["accB"][:, h : h + 1],
                    )
                    # exact part B transposed now; part A in the tail
                    nc.sync.dma_start(
                        out=pt[:, 8 : i + 1, :], in_=ebB[:, :wB],
                        transpose=True,
                    )
                state[i] = st

            def emit_tail(i):
                st = state.pop(i)
                invden = stats.tile([P, HLOC], f32, tag="invden",
                                    name=f"invden{i}")
                if st["late"]:
                    # alpha = exp(mA - m) = exp(negm - negmA), batched [P, 8]
                    dmx = stats.tile([P, HLOC], f32, tag="dmx", name=f"dmx{i}")
                    nc.vector.tensor_tensor(
                        out=dmx, in0=st["negm"], in1=st["negmA"],
                        op=mybir.AluOpType.subtract,
                    )
                    nc.scalar.activation(out=st["alpha"], in_=dmx, func=Exp,
                                         bias=0.0, scale=1.0)
                    # den = alpha * accA + accB
                    nc.vector.tensor_tensor(out=st["accA"], in0=st["accA"],
                                            in1=st["alpha"],
                                            op=mybir.AluOpType.mult)
                    nc.vector.tensor_tensor(out=st["den"], in0=st["accA"],
                                            in1=st["accB"],
                                            op=mybir.AluOpType.add)
                    for h in range(HLOC):
                        nc.vector.tensor_scalar(
                            out=st["ebuf"][h][:, : st["wA"]],
                            in0=st["ebuf"][h][:, : st["wA"]],
                            scalar1=st["alpha"][:, h : h + 1],
                            scalar2=None,
                            op0=mybir.AluOpType.mult,
                        )
                        nc.sync.dma_start(
                            out=st["pts"][h][:, 0:8, :],
                            in_=st["ebuf"][h][:, : st["wA"]],
                            transpose=True,
                        )
                nc.vector.reciprocal(invden, st["den"])

                # PV: po[s, h*64:(h+1)*64] = sum_j E^T_j.T @ v_j
                po = post.tile([P, HD], f32, tag="po", name=f"po{i}")
                for h in range(HLOC):
                    for j in range(i + 1):
                        nc.tensor.matmul(
                            po[:, h * DK : (h + 1) * DK],
                            lhsT=st["pts"][h][:, j, :],
                            rhs=vv[:, j, h * DK : (h + 1) * DK],
                            start=(j == 0),
                            stop=(j == i),
                        )
                # normalize + copy to sbuf in one DVE op
                conc = ccp.tile([P, HD], fp16, tag="conc", name=f"conc{i}")
                nc.vector.tensor_tensor(
                    out=conc[:, :].rearrange("p (h k) -> p h k", h=HLOC),
                    in0=po[:, :].rearrange("p (h k) -> p h k", h=HLOC),
                    in1=invden[:, :, None].broadcast_to((P, HLOC, DK)),
                    op=mybir.AluOpType.mult,
                )
                # conc[s, hd] -> concT[hd%128, m, s-block]
                concT = ccp.tile([P, MPAIRS, P], fp16, tag="concT",
                                 name=f"concT{i}")
                nc.sync.dma_start(out=concT, in_=conc[:, :], transpose=True)
                # output projection for this s-block
                ysb = ccp.tile([P, D], fp16, tag="ysb", name=f"ysb{i}")
                for nch in range(2):
                    ypt = post.tile([P, CH], f32, tag="yp", name=f"yp{i}_{nch}")
                    for m in range(MPAIRS):
                        nc.tensor.matmul(
                            ypt,
                            lhsT=concT[:, m, :],
                            rhs=wo_sb[:, m, nch * CH : (nch + 1) * CH],
                            start=(m == 0),
                            stop=(m == MPAIRS - 1),
                        )
                    nc.scalar.copy(
                        out=ysb[:, nch * CH : (nch + 1) * CH], in_=ypt
                    )
                nc.sync.dma_start(out=y[:][i * P : (i + 1) * P, :], in_=ysb)

            # Block processing order: 1..15 then 0 so the final (drain) tail
            # is the smallest block. Projection chunk work is spread across
            # steps: chunk 0 fully at step 0; chunk nch>=1 split over the
            # three steps before the first block that needs it completes.
            blk_order = list(BLK_ORDER)
            proj_work = {s: [] for s in range(SBLKS)}
            proj_work[0] = [("load", 0)] + [
                ("grp", 0, p, m) for p in ("k", "q") for m in range(MPAIRS)
            ]
            for nch in range(1, 4):
                groups = [("grp", nch, p, m) for p in ("k", "q")
                          for m in range(MPAIRS)]
                steps = PROJ_STEPS[nch]
                proj_work[min(min(steps), 4 * (nch - 1))].append(("load", nch))
                for gi, g in enumerate(groups):
                    proj_work[steps[gi]].append(g)

            for s in range(SBLKS + 1):
                if s < SBLKS:
                    for w in proj_work[s]:
                        if w[0] == "load":
                            emit_kq_load(w[1])
                        else:
                            emit_kq_group(w[1], w[2], w[3])
                    for t2 in range(8):
                        if V_STEPS[t2] == s:
                            emit_v_proj(t2)
                    emit_chains(blk_order[s])
                if s >= 1:
                    emit_tail(blk_order[s - 1])

    nc.finalize()
    return nc


def _prep_inputs(Q, K, V, Wq, Wk, Wv, Wo):
    """Host-side shard + layout prep. Returns list of 8 in_maps."""
    rt8 = math.sqrt(math.sqrt(64.0))  # sqrt(8): scale split over q and k
    tri = np.where(
        np.arange(P)[None, :] <= np.arange(P)[:, None], 0.0, NEG
    ).astype(np.float16)
    ident = np.eye(P, dtype=np.float16)
    in_maps = []
    for c in range(8):
        b, g = c // 2, c % 2
        heads = slice(g * HLOC, (g + 1) * HLOC)
        wq_p = (Wq[heads] * rt8).transpose(1, 0, 2).reshape(D, HD)
        wk_p = (Wk[heads] * rt8).transpose(1, 0, 2).reshape(D, HD)
        wv_p = Wv[heads].transpose(1, 0, 2).reshape(D, HD)
        wo_p = Wo[:, g * HD : (g + 1) * HD].T  # [HD, D]
        in_maps.append({
            "xtq": np.ascontiguousarray(Q[b].T).astype(np.float16),
            "xtk": np.ascontiguousarray(K[b].T).astype(np.float16),
            "xtv": np.ascontiguousarray(V[b].T).astype(np.float16),
            "wq": np.ascontiguousarray(wq_p).astype(np.float16),
            "wk": np.ascontiguousarray(wk_p).astype(np.float16),
            "wv": np.ascontiguousarray(wv_p).astype(np.float16),
            "wo": np.ascontiguousarray(wo_p).astype(np.float16),
            "trimask": tri,
            "ident": ident,
        })
    return in_maps


_NC = []


def kernel(Q, K, V, mask, Wq, Wk, Wv, Wo, bo, _trace=False):
    from concourse.bass_utils import run_bass_kernel_spmd

    Q, K, V = np.asarray(Q), np.asarray(K), np.asarray(V)
    Wq, Wk, Wv = np.asarray(Wq), np.asarray(Wk), np.asarray(Wv)
    Wo, bo = np.asarray(Wo), np.asarray(bo)

    if not _NC:
        _NC.append(build())
    nc = _NC[0]
    in_maps = _prep_inputs(Q, K, V, Wq, Wk, Wv, Wo)
    res = run_bass_kernel_spmd(nc, in_maps, core_ids=list(range(8)), trace=_trace)
    ys = [r["y"].astype(np.float32) for r in res.results]
    out = np.stack([ys[2 * b] + ys[2 * b + 1] for b in range(B)])
    out = out + bo[None, None, :].astype(np.float32)
    if _trace:
        kernel._last = res
    return out.astype(np.float32)


# revision 57
# speedup vs baseline: 1.0759x; 1.0132x over previous
"""Multi-head attention (B=4, S=2048, D=1024, H=16, causal) on 8 trn2 cores.

Sharding: data-parallel over batch (4) x tensor-parallel over head groups (2).
Core c handles batch b=c//2, heads g=c%2 (8 heads each). Each core computes
its partial output projection; host sums the two partials per batch and adds
the bias.

Per-core pipeline (all matmul inputs fp16, fp32 accumulation). The q/k/v
projections are interleaved with the attention blocks (k/q emitted per
512-col chunk right before the four s-blocks that first need them, v per
256-col group during the early blocks) so the projection's PE work overlaps
the attention's DVE/ACT work. Attention per s-block i and local head h uses
an online two-part softmax so score psum tiles are freed early (psum is the
concurrency limiter), software-pipelined one block deep (chains of block i
are emitted before the tail of block i-1 so the PE stream never head-of-line
blocks on the tail's transposes):
  chains(i): scores via K=64 matmuls (causal mask added on the PE via an
    identity-weight matmul of a constant triangular NEG tile); row-max per
    part (DVE); exp(bias=-part max) with accum_out denominator (ACT) ->
    unnormalized E fp16; exact part B transposed to E^T immediately.
  tail(i): part A correction alpha = exp(mA - m) applied to E_A on the
    otherwise-idle gpsimd engine; den = alpha*accA + accB; part A
    transposed; PV in out[s,dk] orientation (po += E^T_j.T @ v_j);
    deferred normalization po * (1/den) fused into the psum->sbuf copy
    (DVE); concat -> concT via DMA transpose; y = concT.T @ Wo -> fp16.
"""

import math

import numpy as np

B, S, D, H = 4, 2048, 1024, 16
DK = 64
HLOC = 8          # heads per core
HD = HLOC * DK    # 512 local concat dims
P = 128
SBLKS = S // P    # 16
CH = 512          # score chunk width
KO = D // P       # 8 contraction tiles for projections
MPAIRS = 4        # head pairs per core
NEG = -30000.0

# schedule/buffer knobs (module-level so they can be tuned)
BLK_ORDER = [2, 3, 4, 5, 6, 7, 8, 9, 10, 11, 15, 14, 13, 12, 0, 1]
EBA_BUFS = 11
EBB_BUFS = 4
# per-chunk list of steps at which the 8 (k,q) m-groups are emitted
PROJ_STEPS = {1: [0, 0, 0, 1, 1, 1, 1, 1],
              2: [3, 3, 4, 4, 5, 5, 6, 6],
              3: [6, 7, 7, 8, 8, 9, 9, 9]}
# valid iff V_STEPS[t2] <= 2*t2-1 (first reader: tail of block 2*t2)
V_STEPS = [1, 1, 2, 3, 4, 5, 6, 7]
YSB_DVE = True   # ysb psum->sbuf copies on DVE instead of ACT
KCOPY_DVE = True  # k-projection psum->sbuf copies on DVE
QCOPY_DVE = False
VCOPY_DVE = False


def build():
    import concourse.bass as bass
    import concourse.mybir as mybir
    import concourse.tile as tile
    from concourse import bacc

    fp16 = mybir.dt.float16
    f32 = mybir.dt.float32

    nc = bacc.Bacc()

    xtq = nc.dram_tensor("xtq", [D, S], fp16, kind="ExternalInput")
    xtk = nc.dram_tensor("xtk", [D, S], fp16, kind="ExternalInput")
    xtv = nc.dram_tensor("xtv", [D, S], fp16, kind="ExternalInput")
    wq = nc.dram_tensor("wq", [D, HD], fp16, kind="ExternalInput")
    wk = nc.dram_tensor("wk", [D, HD], fp16, kind="ExternalInput")
    wv = nc.dram_tensor("wv", [D, HD], fp16, kind="ExternalInput")
    wo = nc.dram_tensor("wo", [HD, D], fp16, kind="ExternalInput")
    trimask = nc.dram_tensor("trimask", [P, P], fp16, kind="ExternalInput")
    ident = nc.dram_tensor("ident", [P, P], fp16, kind="ExternalInput")
    y = nc.dram_tensor("y", [S, D], fp16, kind="ExternalOutput")

    with tile.TileContext(nc) as tc:
        with (
            tc.tile_pool(name="persist", bufs=1) as persist,
            tc.tile_pool(name="stats", bufs=2) as stats,
            tc.tile_pool(name="xkq", bufs=1) as xkqp,
            tc.tile_pool(name="xv", bufs=2) as xvp,
            tc.tile_pool(name="ebA", bufs=EBA_BUFS) as ebAp,
            tc.tile_pool(name="ebB", bufs=EBB_BUFS) as ebBp,
            tc.tile_pool(name="pts", bufs=2) as ptsp,
            tc.tile_pool(name="cc", bufs=2) as ccp,
            tc.tile_pool(name="psp", bufs=3, space="PSUM") as psp,
            tc.tile_pool(name="post", bufs=1, space="PSUM") as post,
        ):
            trimask_sb = persist.tile([P, P], fp16, tag="trimask")
            ident_sb = persist.tile([P, P], fp16, tag="ident")
            wo_sb = persist.tile([P, MPAIRS, D], fp16, tag="wo")
            wq_sb = persist.tile([P, KO, HD], fp16, tag="wq")
            wk_sb = persist.tile([P, KO, HD], fp16, tag="wk")
            wv_sb = persist.tile([P, KO, HD], fp16, tag="wv")
            # k/q weights first: the first projection groups depend on them;
            # wv/wo/masks are needed only a few steps in.
            nc.sync.dma_start(out=wk_sb, in_=wk[:].rearrange("(ko p) n -> p ko n", p=P))
            nc.sync.dma_start(out=wq_sb, in_=wq[:].rearrange("(ko p) n -> p ko n", p=P))
            nc.sync.dma_start(out=trimask_sb, in_=trimask[:])
            nc.sync.dma_start(out=ident_sb, in_=ident[:])
            nc.sync.dma_start(out=wv_sb, in_=wv[:].rearrange("(ko p) n -> p ko n", p=P))
            nc.sync.dma_start(out=wo_sb, in_=wo[:].rearrange("(m p) n -> p m n", p=P))

            # persistent activations
            qt = persist.tile([P, MPAIRS, S], fp16, tag="qt")   # rows = hd % 128
            kt = persist.tile([P, MPAIRS, S], fp16, tag="kt")
            vv = persist.tile([P, SBLKS, HD], fp16, tag="vv")   # [t%128, t//128, hd]

            xq_r = xtq[:].rearrange("(ko p) s -> p ko s", p=P)
            xk_r = xtk[:].rearrange("(ko p) s -> p ko s", p=P)
            xv_r = xtv[:].rearrange("(ko p) s -> p ko s", p=P)

            Exp = mybir.ActivationFunctionType.Exp
            X = mybir.AxisListType.X
            state = {}

            xc_tiles = {}

            def emit_kq_load(nch):
                """DMA one 512-col chunk of X_k / X_q."""
                for src_r, tg in ((xk_r, "xk"), (xq_r, "xq")):
                    xc = xkqp.tile([P, KO, CH], fp16, tag=tg,
                                   name=f"{tg}{nch}")
                    xc_tiles[(tg, nch)] = xc
                    nc.sync.dma_start(
                        out=xc, in_=src_r[:, :, nch * CH : (nch + 1) * CH]
                    )

            def emit_kq_group(nch, proj, m):
                """One m-group of the k or q projection for chunk nch."""
                wsb, dst, tg = ((wk_sb, kt, "xk") if proj == "k"
                                else (wq_sb, qt, "xq"))
                xc = xc_tiles[(tg, nch)]
                ps = psp.tile([P, 2 * CH], f32, tag="ps",
                              name=f"{tg}p{nch}_{m}")
                for ko in range(KO):
                    nc.tensor.matmul(
                        ps[:, :CH],
                        lhsT=wsb[:, ko, m * P : (m + 1) * P],
                        rhs=xc[:, ko, :],
                        start=(ko == 0),
                        stop=(ko == KO - 1),
                    )
                cpy = (nc.vector.tensor_copy
                       if (proj == "k" and KCOPY_DVE) or (proj == "q" and QCOPY_DVE)
                       else nc.scalar.copy)
                cpy(
                    out=dst[:, m, nch * CH : (nch + 1) * CH],
                    in_=ps[:, :CH],
                )

            def emit_v_proj(t2):
                """Project X_v group t2 (two 128-col t-blocks) -> vv."""
                xc = xvp.tile([P, KO, 2 * P], fp16, tag="xv", name=f"xv{t2}")
                nc.sync.dma_start(
                    out=xc, in_=xv_r[:, :, t2 * 2 * P : (t2 + 1) * 2 * P]
                )
                ps = psp.tile([P, 2 * CH], f32, tag="ps", name=f"vp{t2}")
                for half in range(2):
                    for ko in range(KO):
                        nc.tensor.matmul(
                            ps[:, half * CH : (half + 1) * CH],
                            lhsT=xc[:, ko, half * P : (half + 1) * P],
                            rhs=wv_sb[:, ko, :],
                            start=(ko == 0),
                            stop=(ko == KO - 1),
                        )
                eng = nc.vector.tensor_copy if VCOPY_DVE else nc.scalar.copy
                eng(
                    out=vv[:, t2 * 2 : t2 * 2 + 2, :].rearrange("p a b -> p (a b)"),
                    in_=ps,
                )

            def emit_chains(i):
                c, r = i // 4, i % 4
                width = (i + 1) * P
                late = c >= 2
                wA = min(width, 2 * CH)
                wB = width - wA
                st = {"late": late, "wA": wA, "wB": wB, "pts": [], "ebuf": []}
                st["den"] = stats.tile([P, HLOC], f32, tag="den", name=f"den{i}")
                st["negmA"] = stats.tile([P, HLOC], f32, tag="negmA",
                                         name=f"negmA{i}")
                if late:
                    for t in ("negmB", "negm", "accA", "accB", "alpha"):
                        st[t] = stats.tile([P, HLOC], f32, tag=t, name=f"{t}{i}")
                for h in range(HLOC):
                    m, z = h // 2, h % 2
                    qts = qt[z * DK : (z + 1) * DK, m, i * P : (i + 1) * P]
                    tA = psp.tile([P, 2 * CH], f32, tag="ps", name=f"sA{i}_{h}")
                    for cc in range(min(c, 1) + 1):
                        w = CH if cc < c else (r + 1) * P
                        nc.tensor.matmul(
                            tA[:, cc * CH : cc * CH + w],
                            lhsT=qts,
                            rhs=kt[z * DK : (z + 1) * DK, m, cc * CH : cc * CH + w],
                            start=True,
                            stop=(cc != c),
                        )
                    if not late:
                        nc.tensor.matmul(
                            tA[:, c * CH + r * P : c * CH + (r + 1) * P],
                            lhsT=ident_sb, rhs=trimask_sb,
                            start=False, stop=True,
                        )
                    nc.vector.reduce_max(
                        st["negmA"][:, h : h + 1], tA[:, :wA], axis=X,
                        negate=True,
                    )
                    ebuf = ebAp.tile([P, 2 * CH], fp16, tag="eb",
                                     name=f"eb{i}_{h}")
                    st["ebuf"].append(ebuf)
                    pt = ptsp.tile([P, SBLKS, P], fp16, tag=f"pt{h}",
                                   name=f"pt{i}_{h}")
                    st["pts"].append(pt)
                    nc.scalar.activation(
                        out=ebuf[:, :wA], in_=tA[:, :wA], func=Exp,
                        bias=st["negmA"][:, h : h + 1], scale=1.0,
                        accum_out=(st["accA"] if late else st["den"])[:, h : h + 1],
                    )
                    if not late:
                        nc.sync.dma_start(
                            out=pt[:, 0 : i + 1, :], in_=ebuf[:, 0:width],
                            transpose=True,
                        )
                        continue
                    tB = psp.tile([P, 2 * CH], f32, tag="ps", name=f"sB{i}_{h}")
                    for cc in range(2, c + 1):
                        w = CH if cc < c else (r + 1) * P
                        nc.tensor.matmul(
                            tB[:, (cc - 2) * CH : (cc - 2) * CH + w],
                            lhsT=qts,
                            rhs=kt[z * DK : (z + 1) * DK, m, cc * CH : cc * CH + w],
                            start=True,
                            stop=(cc != c),
                        )
                    nc.tensor.matmul(
                        tB[:, (c - 2) * CH + r * P : (c - 2) * CH + (r + 1) * P],
                        lhsT=ident_sb, rhs=trimask_sb,
                        start=False, stop=True,
                    )
                    nc.vector.reduce_max(
                        st["negmB"][:, h : h + 1], tB[:, :wB], axis=X,
                        negate=True,
                    )
                    # negm = -max(mA, mB) = min(negmA, negmB)
                    nc.vector.tensor_tensor(
                        out=st["negm"][:, h : h + 1],
                        in0=st["negmA"][:, h : h + 1],
                        in1=st["negmB"][:, h : h + 1], op=mybir.AluOpType.min,
                    )
                    ebB = ebBp.tile([P, 2 * CH], fp16, tag="ebB",
                                    name=f"ebB{i}_{h}")
                    nc.scalar.activation(
                        out=ebB[:, :wB], in_=tB[:, :wB], func=Exp,
                        bias=st["negm"][:, h : h + 1], scale=1.0,
                        accum_out=st["accB"][:, h : h + 1],
                    )
                    # exact part B transposed now; part A in the tail
                    nc.sync.dma_start(
                        out=pt[:, 8 : i + 1, :], in_=ebB[:, :wB],
                        transpose=True,
                    )
                state[i] = st

            def emit_tail(i):
                st = state.pop(i)
                invden = stats.tile([P, HLOC], f32, tag="invden",
                                    name=f"invden{i}")
                if st["late"]:
                    # alpha = exp(mA - m) = exp(negm - negmA), batched [P, 8]
                    dmx = stats.tile([P, HLOC], f32, tag="dmx", name=f"dmx{i}")
                    nc.vector.tensor_tensor(
                        out=dmx, in0=st["negm"], in1=st["negmA"],
                        op=mybir.AluOpType.subtract,
                    )
                    nc.scalar.activation(out=st["alpha"], in_=dmx, func=Exp,
                                         bias=0.0, scale=1.0)
                    # den = alpha * accA + accB
                    nc.vector.tensor_tensor(out=st["accA"], in0=st["accA"],
                                            in1=st["alpha"],
                                            op=mybir.AluOpType.mult)
                    nc.vector.tensor_tensor(out=st["den"], in0=st["accA"],
                                            in1=st["accB"],
                                            op=mybir.AluOpType.add)
                    for h in range(HLOC):
                        nc.vector.tensor_scalar(
                            out=st["ebuf"][h][:, : st["wA"]],
                            in0=st["ebuf"][h][:, : st["wA"]],
                            scalar1=st["alpha"][:, h : h + 1],
                            scalar2=None,
                            op0=mybir.AluOpType.mult,
                        )
                        nc.sync.dma_start(
                            out=st["pts"][h][:, 0:8, :],
                            in_=st["ebuf"][h][:, : st["wA"]],
                            transpose=True,
                        )
                nc.vector.reciprocal(invden, st["den"])

                # PV: po[s, h*64:(h+1)*64] = sum_j E^T_j.T @ v_j
                po = post.tile([P, HD], f32, tag="po", name=f"po{i}")
                for h in range(HLOC):
                    for j in range(i + 1):
                        nc.tensor.matmul(
                            po[:, h * DK : (h + 1) * DK],
                            lhsT=st["pts"][h][:, j, :],
                            rhs=vv[:, j, h * DK : (h + 1) * DK],
                            start=(j == 0),
                            stop=(j == i),
                        )
                # normalize + copy to sbuf in one DVE op
                conc = ccp.tile([P, HD], fp16, tag="conc", name=f"conc{i}")
                nc.vector.tensor_tensor(
                    out=conc[:, :].rearrange("p (h k) -> p h k", h=HLOC),
                    in0=po[:, :].rearrange("p (h k) -> p h k", h=HLOC),
                    in1=invden[:, :, None].broadcast_to((P, HLOC, DK)),
                    op=mybir.AluOpType.mult,
                )
                # conc[s, hd] -> concT[hd%128, m, s-block]
                concT = ccp.tile([P, MPAIRS, P], fp16, tag="concT",
                                 name=f"concT{i}")
                nc.sync.dma_start(out=concT, in_=conc[:, :], transpose=True)
                # output projection for this s-block
                ysb = ccp.tile([P, D], fp16, tag="ysb", name=f"ysb{i}")
                for nch in range(2):
                    ypt = post.tile([P, CH], f32, tag="yp", name=f"yp{i}_{nch}")
                    for m in range(MPAIRS):
                        nc.tensor.matmul(
                            ypt,
                            lhsT=concT[:, m, :],
                            rhs=wo_sb[:, m, nch * CH : (nch + 1) * CH],
                            start=(m == 0),
                            stop=(m == MPAIRS - 1),
                        )
                    ycpy = nc.vector.tensor_copy if YSB_DVE else nc.scalar.copy
                    ycpy(
                        out=ysb[:, nch * CH : (nch + 1) * CH], in_=ypt
                    )
                nc.sync.dma_start(out=y[:][i * P : (i + 1) * P, :], in_=ysb)

            # Block processing order: 1..15 then 0 so the final (drain) tail
            # is the smallest block. Projection chunk work is spread across
            # steps: chunk 0 fully at step 0; chunk nch>=1 split over the
            # three steps before the first block that needs it completes.
            blk_order = list(BLK_ORDER)
            proj_work = {s: [] for s in range(SBLKS)}
            proj_work[0] = [("load", 0)] + [
                ("grp", 0, p, m) for p in ("k", "q") for m in range(MPAIRS)
            ]
            for nch in range(1, 4):
                groups = [("grp", nch, p, m) for p in ("k", "q")
                          for m in range(MPAIRS)]
                steps = PROJ_STEPS[nch]
                proj_work[min(min(steps), 4 * (nch - 1))].append(("load", nch))
                for gi, g in enumerate(groups):
                    proj_work[steps[gi]].append(g)

            for s in range(SBLKS + 1):
                if s < SBLKS:
                    for w in proj_work[s]:
                        if w[0] == "load":
                            emit_kq_load(w[1])
                        else:
                            emit_kq_group(w[1], w[2], w[3])
                    for t2 in range(8):
                        if V_STEPS[t2] == s:
                            emit_v_proj(t2)
                    emit_chains(blk_order[s])
                if s >= 1:
                    emit_tail(blk_order[s - 1])

    nc.finalize()
    return nc


def _prep_inputs(Q, K, V, Wq, Wk, Wv, Wo):
    """Host-side shard + layout prep. Returns list of 8 in_maps."""
    rt8 = math.sqrt(math.sqrt(64.0))  # sqrt(8): scale split over q and k
    tri = np.where(
        np.arange(P)[None, :] <= np.arange(P)[:, None], 0.0, NEG
    ).astype(np.float16)
    ident = np.eye(P, dtype=np.float16)
    in_maps = []
    for c in range(8):
        b, g = c // 2, c % 2
        heads = slice(g * HLOC, (g + 1) * HLOC)
        wq_p = (Wq[heads] * rt8).transpose(1, 0, 2).reshape(D, HD)
        wk_p = (Wk[heads] * rt8).transpose(1, 0, 2).reshape(D, HD)
        wv_p = Wv[heads].transpose(1, 0, 2).reshape(D, HD)
        wo_p = Wo[:, g * HD : (g + 1) * HD].T  # [HD, D]
        in_maps.append({
            "xtq": np.ascontiguousarray(Q[b].T).astype(np.float16),
            "xtk": np.ascontiguousarray(K[b].T).astype(np.float16),
            "xtv": np.ascontiguousarray(V[b].T).astype(np.float16),
            "wq": np.ascontiguousarray(wq_p).astype(np.float16),
            "wk": np.ascontiguousarray(wk_p).astype(np.float16),
            "wv": np.ascontiguousarray(wv_p).astype(np.float16),
            "wo": np.ascontiguousarray(wo_p).astype(np.float16),
            "trimask": tri,
            "ident": ident,
        })
    return in_maps


_NC = []


def kernel(Q, K, V, mask, Wq, Wk, Wv, Wo, bo, _trace=False):
    from concourse.bass_utils import run_bass_kernel_spmd

    Q, K, V = np.asarray(Q), np.asarray(K), np.asarray(V)
    Wq, Wk, Wv = np.asarray(Wq), np.asarray(Wk), np.asarray(Wv)
    Wo, bo = np.asarray(Wo), np.asarray(bo)

    if not _NC:
        _NC.append(build())
    nc = _NC[0]
    in_maps = _prep_inputs(Q, K, V, Wq, Wk, Wv, Wo)
    res = run_bass_kernel_spmd(nc, in_maps, core_ids=list(range(8)), trace=_trace)
    ys = [r["y"].astype(np.float32) for r in res.results]
    out = np.stack([ys[2 * b] + ys[2 * b + 1] for b in range(B)])
    out = out + bo[None, None, :].astype(np.float32)
    if _trace:
        kernel._last = res
    return out.astype(np.float32)


# revision 77
# speedup vs baseline: 1.0956x; 1.0183x over previous
"""Multi-head attention (B=4, S=2048, D=1024, H=16, causal) on 8 trn2 cores.

Sharding: data-parallel over batch (4) x tensor-parallel over head groups (2).
Core c handles batch b=c//2, heads g=c%2 (8 heads each). Each core computes
its partial output projection; host sums the two partials per batch and adds
the bias.

Per-core pipeline (all matmul inputs fp16, fp32 accumulation). The q/k/v
projections are interleaved with the attention blocks (k/q emitted per
512-col chunk right before the four s-blocks that first need them, v per
256-col group during the early blocks) so the projection's PE work overlaps
the attention's DVE/ACT work. Attention per s-block i and local head h uses
an online two-part softmax so score psum tiles are freed early (psum is the
concurrency limiter), software-pipelined one block deep (chains of block i
are emitted before the tail of block i-1 so the PE stream never head-of-line
blocks on the tail's transposes):
  chains(i): scores via K=64 matmuls (causal mask added on the PE via an
    identity-weight matmul of a constant triangular NEG tile); row-max per
    part (DVE); exp(bias=-part max) with accum_out denominator (ACT) ->
    unnormalized E fp16; exact part B transposed to E^T immediately.
  tail(i): part A correction alpha = exp(mA - m) applied to E_A on the
    otherwise-idle gpsimd engine; den = alpha*accA + accB; part A
    transposed; PV in out[s,dk] orientation (po += E^T_j.T @ v_j);
    deferred normalization po * (1/den) fused into the psum->sbuf copy
    (DVE); concat -> concT via DMA transpose; y = concT.T @ Wo -> fp16.
"""

import math

import numpy as np

B, S, D, H = 4, 2048, 1024, 16
DK = 64
HLOC = 8          # heads per core
HD = HLOC * DK    # 512 local concat dims
P = 128
SBLKS = S // P    # 16
CH = 512          # score chunk width
KO = D // P       # 8 contraction tiles for projections
MPAIRS = 4        # head pairs per core
NEG = -30000.0

# schedule/buffer knobs (module-level so they can be tuned)
BLK_ORDER = [2, 3, 4, 5, 6, 7, 8, 9, 10, 11, 15, 14, 13, 12, 0, 1]
EBA_BUFS = 11
EBB_BUFS = 4
# per-chunk list of steps at which the 8 (k,q) m-groups are emitted
PROJ_STEPS = {1: [0, 0, 0, 1, 1, 1, 1, 1],
              2: [3, 3, 4, 4, 5, 5, 6, 6],
              3: [6, 7, 7, 8, 8, 9, 9, 9]}
# valid iff V_STEPS[t2] <= 2*t2-1 (first reader: tail of block 2*t2)
V_STEPS = [1, 1, 2, 4, 5, 6, 7, 8]
YSB_DVE = True   # ysb psum->sbuf copies on DVE instead of ACT
KCOPY_DVE = True  # k-projection psum->sbuf copies on DVE
QCOPY_DVE = False
STATS_BUFS = 2
VCOPY_DVE = False


def build():
    import concourse.bass as bass
    import concourse.mybir as mybir
    import concourse.tile as tile
    from concourse import bacc

    fp16 = mybir.dt.float16
    f32 = mybir.dt.float32

    nc = bacc.Bacc()

    xtq = nc.dram_tensor("xtq", [D, S], fp16, kind="ExternalInput")
    xtk = nc.dram_tensor("xtk", [D, S], fp16, kind="ExternalInput")
    xtv = nc.dram_tensor("xtv", [D, S], fp16, kind="ExternalInput")
    wq = nc.dram_tensor("wq", [D, HD], fp16, kind="ExternalInput")
    wk = nc.dram_tensor("wk", [D, HD], fp16, kind="ExternalInput")
    wv = nc.dram_tensor("wv", [D, HD], fp16, kind="ExternalInput")
    wo = nc.dram_tensor("wo", [HD, D], fp16, kind="ExternalInput")
    trimask = nc.dram_tensor("trimask", [P, P], fp16, kind="ExternalInput")
    ident = nc.dram_tensor("ident", [P, P], fp16, kind="ExternalInput")
    y = nc.dram_tensor("y", [S, D], fp16, kind="ExternalOutput")

    with tile.TileContext(nc) as tc:
        with (
            tc.tile_pool(name="persist", bufs=1) as persist,
            tc.tile_pool(name="stats", bufs=STATS_BUFS) as stats,
            tc.tile_pool(name="xkq", bufs=1) as xkqp,
            tc.tile_pool(name="xv", bufs=2) as xvp,
            tc.tile_pool(name="ebA", bufs=EBA_BUFS) as ebAp,
            tc.tile_pool(name="ebB", bufs=EBB_BUFS) as ebBp,
            tc.tile_pool(name="pts", bufs=2) as ptsp,
            tc.tile_pool(name="cc", bufs=2) as ccp,
            tc.tile_pool(name="psp", bufs=3, space="PSUM") as psp,
            tc.tile_pool(name="post", bufs=1, space="PSUM") as post,
        ):
            trimask_sb = persist.tile([P, P], fp16, tag="trimask")
            ident_sb = persist.tile([P, P], fp16, tag="ident")
            wo_sb = persist.tile([P, MPAIRS, D], fp16, tag="wo")
            wq_sb = persist.tile([P, KO, HD], fp16, tag="wq")
            wk_sb = persist.tile([P, KO, HD], fp16, tag="wk")
            wv_sb = persist.tile([P, KO, HD], fp16, tag="wv")
            # k/q weights first: the first projection groups depend on them;
            # wv/wo/masks are needed only a few steps in.
            nc.sync.dma_start(out=wk_sb, in_=wk[:].rearrange("(ko p) n -> p ko n", p=P))
            nc.sync.dma_start(out=wq_sb, in_=wq[:].rearrange("(ko p) n -> p ko n", p=P))
            nc.sync.dma_start(out=trimask_sb, in_=trimask[:])
            nc.sync.dma_start(out=ident_sb, in_=ident[:])
            nc.sync.dma_start(out=wv_sb, in_=wv[:].rearrange("(ko p) n -> p ko n", p=P))
            nc.sync.dma_start(out=wo_sb, in_=wo[:].rearrange("(m p) n -> p m n", p=P))

            # persistent activations
            qt = persist.tile([P, MPAIRS, S], fp16, tag="qt")   # rows = hd % 128
            kt = persist.tile([P, MPAIRS, S], fp16, tag="kt")
            vv = persist.tile([P, SBLKS, HD], fp16, tag="vv")   # [t%128, t//128, hd]

            xq_r = xtq[:].rearrange("(ko p) s -> p ko s", p=P)
            xk_r = xtk[:].rearrange("(ko p) s -> p ko s", p=P)
            xv_r = xtv[:].rearrange("(ko p) s -> p ko s", p=P)

            Exp = mybir.ActivationFunctionType.Exp
            X = mybir.AxisListType.X
            state = {}

            xc_tiles = {}

            def emit_kq_load(nch):
                """DMA one 512-col chunk of X_k / X_q."""
                for src_r, tg in ((xk_r, "xk"), (xq_r, "xq")):
                    xc = xkqp.tile([P, KO, CH], fp16, tag=tg,
                                   name=f"{tg}{nch}")
                    xc_tiles[(tg, nch)] = xc
                    nc.sync.dma_start(
                        out=xc, in_=src_r[:, :, nch * CH : (nch + 1) * CH]
                    )

            def emit_kq_group(nch, proj, m, ps, half):
                """One m-group of the k or q projection for chunk nch,
                using one 512-col bank of a shared 2-bank psum tile."""
                wsb, dst, tg = ((wk_sb, kt, "xk") if proj == "k"
                                else (wq_sb, qt, "xq"))
                xc = xc_tiles[(tg, nch)]
                for ko in range(KO):
                    nc.tensor.matmul(
                        ps[:, half * CH : (half + 1) * CH],
                        lhsT=wsb[:, ko, m * P : (m + 1) * P],
                        rhs=xc[:, ko, :],
                        start=(ko == 0),
                        stop=(ko == KO - 1),
                    )
                cpy = (nc.vector.tensor_copy
                       if (proj == "k" and KCOPY_DVE) or (proj == "q" and QCOPY_DVE)
                       else nc.scalar.copy)
                cpy(
                    out=dst[:, m, nch * CH : (nch + 1) * CH],
                    in_=ps[:, half * CH : (half + 1) * CH],
                )

            def emit_v_proj(t2):
                """Project X_v group t2 (two 128-col t-blocks) -> vv."""
                xc = xvp.tile([P, KO, 2 * P], fp16, tag="xv", name=f"xv{t2}")
                nc.sync.dma_start(
                    out=xc, in_=xv_r[:, :, t2 * 2 * P : (t2 + 1) * 2 * P]
                )
                ps = psp.tile([P, 2 * CH], f32, tag="ps", name=f"vp{t2}")
                for half in range(2):
                    for ko in range(KO):
                        nc.tensor.matmul(
                            ps[:, half * CH : (half + 1) * CH],
                            lhsT=xc[:, ko, half * P : (half + 1) * P],
                            rhs=wv_sb[:, ko, :],
                            start=(ko == 0),
                            stop=(ko == KO - 1),
                        )
                eng = nc.vector.tensor_copy if VCOPY_DVE else nc.scalar.copy
                eng(
                    out=vv[:, t2 * 2 : t2 * 2 + 2, :].rearrange("p a b -> p (a b)"),
                    in_=ps,
                )

            def emit_chains(i):
                c, r = i // 4, i % 4
                width = (i + 1) * P
                late = c >= 2
                wA = min(width, 2 * CH)
                wB = width - wA
                st = {"late": late, "wA": wA, "wB": wB, "pts": [], "ebuf": []}
                st["den"] = stats.tile([P, HLOC], f32, tag="den", name=f"den{i}")
                st["negmA"] = stats.tile([P, HLOC], f32, tag="negmA",
                                         name=f"negmA{i}")
                if late:
                    for t in ("negmB", "negm", "accA", "accB", "alpha"):
                        st[t] = stats.tile([P, HLOC], f32, tag=t, name=f"{t}{i}")
                for h in range(HLOC):
                    m, z = h // 2, h % 2
                    qts = qt[z * DK : (z + 1) * DK, m, i * P : (i + 1) * P]
                    tA = psp.tile([P, 2 * CH], f32, tag="ps", name=f"sA{i}_{h}")
                    for cc in range(min(c, 1) + 1):
                        w = CH if cc < c else (r + 1) * P
                        nc.tensor.matmul(
                            tA[:, cc * CH : cc * CH + w],
                            lhsT=qts,
                            rhs=kt[z * DK : (z + 1) * DK, m, cc * CH : cc * CH + w],
                            start=True,
                            stop=(cc != c),
                        )
                    if not late:
                        nc.tensor.matmul(
                            tA[:, c * CH + r * P : c * CH + (r + 1) * P],
                            lhsT=ident_sb, rhs=trimask_sb,
                            start=False, stop=True,
                        )
                    nc.vector.reduce_max(
                        st["negmA"][:, h : h + 1], tA[:, :wA], axis=X,
                        negate=True,
                    )
                    ebuf = ebAp.tile([P, 2 * CH], fp16, tag="eb",
                                     name=f"eb{i}_{h}")
                    st["ebuf"].append(ebuf)
                    pt = ptsp.tile([P, SBLKS, P], fp16, tag=f"pt{h}",
                                   name=f"pt{i}_{h}")
                    st["pts"].append(pt)
                    nc.scalar.activation(
                        out=ebuf[:, :wA], in_=tA[:, :wA], func=Exp,
                        bias=st["negmA"][:, h : h + 1], scale=1.0,
                        accum_out=(st["accA"] if late else st["den"])[:, h : h + 1],
                    )
                    if not late:
                        nc.sync.dma_start(
                            out=pt[:, 0 : i + 1, :], in_=ebuf[:, 0:width],
                            transpose=True,
                        )
                        continue
                    tB = psp.tile([P, 2 * CH], f32, tag="ps", name=f"sB{i}_{h}")
                    for cc in range(2, c + 1):
                        w = CH if cc < c else (r + 1) * P
                        nc.tensor.matmul(
                            tB[:, (cc - 2) * CH : (cc - 2) * CH + w],
                            lhsT=qts,
                            rhs=kt[z * DK : (z + 1) * DK, m, cc * CH : cc * CH + w],
                            start=True,
                            stop=(cc != c),
                        )
                    nc.tensor.matmul(
                        tB[:, (c - 2) * CH + r * P : (c - 2) * CH + (r + 1) * P],
                        lhsT=ident_sb, rhs=trimask_sb,
                        start=False, stop=True,
                    )
                    nc.vector.reduce_max(
                        st["negmB"][:, h : h + 1], tB[:, :wB], axis=X,
                        negate=True,
                    )
                    # negm = -max(mA, mB) = min(negmA, negmB)
                    nc.vector.tensor_tensor(
                        out=st["negm"][:, h : h + 1],
                        in0=st["negmA"][:, h : h + 1],
                        in1=st["negmB"][:, h : h + 1], op=mybir.AluOpType.min,
                    )
                    ebB = ebBp.tile([P, 2 * CH], fp16, tag="ebB",
                                    name=f"ebB{i}_{h}")
                    nc.scalar.activation(
                        out=ebB[:, :wB], in_=tB[:, :wB], func=Exp,
                        bias=st["negm"][:, h : h + 1], scale=1.0,
                        accum_out=st["accB"][:, h : h + 1],
                    )
                    # exact part B transposed now; part A in the tail
                    nc.sync.dma_start(
                        out=pt[:, 8 : i + 1, :], in_=ebB[:, :wB],
                        transpose=True,
                    )
                state[i] = st

            def emit_tail(i):
                st = state.pop(i)
                invden = stats.tile([P, HLOC], f32, tag="invden",
                                    name=f"invden{i}")
                if st["late"]:
                    # alpha = exp(mA - m) = exp(negm - negmA), batched [P, 8]
                    dmx = stats.tile([P, HLOC], f32, tag="dmx", name=f"dmx{i}")
                    nc.vector.tensor_tensor(
                        out=dmx, in0=st["negm"], in1=st["negmA"],
                        op=mybir.AluOpType.subtract,
                    )
                    nc.scalar.activation(out=st["alpha"], in_=dmx, func=Exp,
                                         bias=0.0, scale=1.0)
                    # den = alpha * accA + accB
                    nc.vector.tensor_tensor(out=st["accA"], in0=st["accA"],
                                            in1=st["alpha"],
                                            op=mybir.AluOpType.mult)
                    nc.vector.tensor_tensor(out=st["den"], in0=st["accA"],
                                            in1=st["accB"],
                                            op=mybir.AluOpType.add)
                    for h in range(HLOC):
                        nc.vector.tensor_scalar(
                            out=st["ebuf"][h][:, : st["wA"]],
                            in0=st["ebuf"][h][:, : st["wA"]],
                            scalar1=st["alpha"][:, h : h + 1],
                            scalar2=None,
                            op0=mybir.AluOpType.mult,
                        )
                        nc.sync.dma_start(
                            out=st["pts"][h][:, 0:8, :],
                            in_=st["ebuf"][h][:, : st["wA"]],
                            transpose=True,
                        )
                nc.vector.reciprocal(invden, st["den"])

                # PV: po[s, h*64:(h+1)*64] = sum_j E^T_j.T @ v_j.
                # B-part t-blocks (transposed during chains) accumulate first
                # so PV starts before the alpha-gated part-A transposes land.
                po = post.tile([P, HD], f32, tag="po", name=f"po{i}")
                jorder = (list(range(8, i + 1)) + list(range(8))
                          if st["late"] else list(range(i + 1)))
                for h in range(HLOC):
                    for jn, j in enumerate(jorder):
                        nc.tensor.matmul(
                            po[:, h * DK : (h + 1) * DK],
                            lhsT=st["pts"][h][:, j, :],
                            rhs=vv[:, j, h * DK : (h + 1) * DK],
                            start=(jn == 0),
                            stop=(jn == len(jorder) - 1),
                        )
                # normalize + copy to sbuf in one DVE op
                conc = ccp.tile([P, HD], fp16, tag="conc", name=f"conc{i}")
                nc.vector.tensor_tensor(
                    out=conc[:, :].rearrange("p (h k) -> p h k", h=HLOC),
                    in0=po[:, :].rearrange("p (h k) -> p h k", h=HLOC),
                    in1=invden[:, :, None].broadcast_to((P, HLOC, DK)),
                    op=mybir.AluOpType.mult,
                )
                # conc[s, hd] -> concT[hd%128, m, s-block]
                concT = ccp.tile([P, MPAIRS, P], fp16, tag="concT",
                                 name=f"concT{i}")
                nc.sync.dma_start(out=concT, in_=conc[:, :], transpose=True)
                # output projection for this s-block
                ysb = ccp.tile([P, D], fp16, tag="ysb", name=f"ysb{i}")
                for nch in range(2):
                    ypt = post.tile([P, CH], f32, tag="yp", name=f"yp{i}_{nch}")
                    for m in range(MPAIRS):
                        nc.tensor.matmul(
                            ypt,
                            lhsT=concT[:, m, :],
                            rhs=wo_sb[:, m, nch * CH : (nch + 1) * CH],
                            start=(m == 0),
                            stop=(m == MPAIRS - 1),
                        )
                    ycpy = nc.vector.tensor_copy if YSB_DVE else nc.scalar.copy
                    ycpy(
                        out=ysb[:, nch * CH : (nch + 1) * CH], in_=ypt
                    )
                nc.sync.dma_start(out=y[:][i * P : (i + 1) * P, :], in_=ysb)

            # Block processing order: 1..15 then 0 so the final (drain) tail
            # is the smallest block. Projection chunk work is spread across
            # steps: chunk 0 fully at step 0; chunk nch>=1 split over the
            # three steps before the first block that needs it completes.
            blk_order = list(BLK_ORDER)
            proj_work = {s: [] for s in range(SBLKS)}
            proj_work[0] = [("load", 0)] + [
                ("grp", 0, p, m) for p in ("k", "q") for m in range(MPAIRS)
            ]
            for nch in range(1, 4):
                groups = [("grp", nch, p, m) for p in ("k", "q")
                          for m in range(MPAIRS)]
                steps = PROJ_STEPS[nch]
                proj_work[min(min(steps), 4 * (nch - 1))].append(("load", nch))
                for gi, g in enumerate(groups):
                    proj_work[steps[gi]].append(g)

            for s in range(SBLKS + 1):
                if s < SBLKS:
                    grps = [w for w in proj_work[s] if w[0] == "grp"]
                    for w in proj_work[s]:
                        if w[0] == "load":
                            emit_kq_load(w[1])
                    for gi in range(0, len(grps), 2):
                        pair = grps[gi : gi + 2]
                        ps = psp.tile([P, 2 * CH], f32, tag="ps",
                                      name=f"pp{s}_{gi}")
                        for half, w in enumerate(pair):
                            emit_kq_group(w[1], w[2], w[3], ps, half)
                    for t2 in range(8):
                        if V_STEPS[t2] == s:
                            emit_v_proj(t2)
                    emit_chains(blk_order[s])
                if s >= 1:
                    emit_tail(blk_order[s - 1])

    nc.finalize()
    return nc


def _prep_inputs(Q, K, V, Wq, Wk, Wv, Wo):
    """Host-side shard + layout prep. Returns list of 8 in_maps."""
    rt8 = math.sqrt(math.sqrt(64.0))  # sqrt(8): scale split over q and k
    tri = np.where(
        np.arange(P)[None, :] <= np.arange(P)[:, None], 0.0, NEG
    ).astype(np.float16)
    ident = np.eye(P, dtype=np.float16)
    in_maps = []
    for c in range(8):
        b, g = c // 2, c % 2
        heads = slice(g * HLOC, (g + 1) * HLOC)
        wq_p = (Wq[heads] * rt8).transpose(1, 0, 2).reshape(D, HD)
        wk_p = (Wk[heads] * rt8).transpose(1, 0, 2).reshape(D, HD)
        wv_p = Wv[heads].transpose(1, 0, 2).reshape(D, HD)
        wo_p = Wo[:, g * HD : (g + 1) * HD].T  # [HD, D]
        in_maps.append({
            "xtq": np.ascontiguousarray(Q[b].T).astype(np.float16),
            "xtk": np.ascontiguousarray(K[b].T).astype(np.float16),
            "xtv": np.ascontiguousarray(V[b].T).astype(np.float16),
            "wq": np.ascontiguousarray(wq_p).astype(np.float16),
            "wk": np.ascontiguousarray(wk_p).astype(np.float16),
            "wv": np.ascontiguousarray(wv_p).astype(np.float16),
            "wo": np.ascontiguousarray(wo_p).astype(np.float16),
            "trimask": tri,
            "ident": ident,
        })
    return in_maps


_NC = []


def kernel(Q, K, V, mask, Wq, Wk, Wv, Wo, bo, _trace=False):
    from concourse.bass_utils import run_bass_kernel_spmd

    Q, K, V = np.asarray(Q), np.asarray(K), np.asarray(V)
    Wq, Wk, Wv = np.asarray(Wq), np.asarray(Wk), np.asarray(Wv)
    Wo, bo = np.asarray(Wo), np.asarray(bo)

    if not _NC:
        _NC.append(build())
    nc = _NC[0]
    in_maps = _prep_inputs(Q, K, V, Wq, Wk, Wv, Wo)
    res = run_bass_kernel_spmd(nc, in_maps, core_ids=list(range(8)), trace=_trace)
    ys = [r["y"].astype(np.float32) for r in res.results]
    out = np.stack([ys[2 * b] + ys[2 * b + 1] for b in range(B)])
    out = out + bo[None, None, :].astype(np.float32)
    if _trace:
        kernel._last = res
    return out.astype(np.float32)
